# revision 23
# baseline (speedup 1.0000x reference)
"""GCN encoder (3-layer GCNConv + BatchNorm + ReLU + global mean pool) on 8
Trainium2 NeuronCores.

Strategy (graph/data parallel, edges sharded by destination):
  - Nodes are split into 8 contiguous shards (one per core). Each core owns
    all edges whose destination lands in its shard.
  - The layer is computed aggregate-first (mathematically identical to the
    reference's transform-first order since GCNConv is linear):
        z[v]  = sum_{e: dst=v} w_e * h[src_e] + snorm_v * h[v]
        hpre  = W.T @ z                          (kept transposed: [D, nodes])
        h_out = relu(gamma * (hpre - mu) / sqrt(var+eps) + beta)
  - The gather h[src_e] uses dma_gather (int16 indices, 2048 rows per
    instruction) from a replicated full node table in HBM. Since int16
    limits the index range, edges are split into two phases by source half
    (lo: src < N/2, hi: src >= N/2 gathered from an offset table view).
  - Per 128-edge tile, a selection matrix Sel[e, dst_local] = w_e is built
    on-chip with one tensor_scalar(is_equal, mult) against an iota constant,
    and the scatter-add becomes a PE matmul G.T @ Sel accumulated in PSUM
    over a 128-destination window. Self-loops are a diagonal-matrix matmul
    against the previous layer's activations already in SBUF.
  - BatchNorm statistics are free-axis reductions in the transposed layout;
    partials are combined with a [128,2] AllReduce. After normalization the
    result is transposed back (PE transpose) and AllGathered into the next
    layer's node table.
  - Mean pooling reuses the selection-matmul trick against the sorted graph
    ids, followed by a [128,256] AllReduce and division by counts.
"""

import sys

sys.path.insert(0, "/opt/trn_rl_repo")

import numpy as np

import concourse.bass as bass
import concourse.tile as tile
from concourse import bacc, mybir
from concourse import bass_utils
from concourse.masks import make_identity

F32 = mybir.dt.float32
I16 = mybir.dt.int16
OP = mybir.AluOpType
ACTF = mybir.ActivationFunctionType

NCORES = 8
D = 128
P = 128
GB_TILES = 16     # 128-edge tiles per dma_gather
WBLK = 512        # node columns per W-matmul / BN block
EPS = 1e-5


class Cfg:
    def __init__(self, N, E, G, L=3):
        assert N % NCORES == 0 and N % 2 == 0
        self.N, self.E, self.G, self.L = N, E, G, L
        self.HALF = N // 2
        self.NP = N // NCORES                    # nodes per core
        self.nwin = -(-self.NP // P)             # 128-dst windows per core
        self.winlens = [min(P, self.NP - w * P) for w in range(self.nwin)]
        self.nblk = -(-self.NP // WBLK)          # 512-node BN/W blocks
        self.blens = [min(WBLK, self.NP - b * WBLK) for b in range(self.nblk)]
        self.nfull = self.NP // P                # full 128-node tiles
        self.rem = self.NP - self.nfull * P
        self.gblk = -(-G // P)                   # 128-graph output tiles
        assert self.gblk * P == G or G <= P


def host_preprocess(cfg, x, edge_index, batch, Ws, bs, gammas, betas):
    """Shard + sort edges, build per-core packed metadata arrays."""
    N, G = cfg.N, cfg.G
    NP, HALF = cfg.NP, cfg.HALF
    x = np.ascontiguousarray(np.asarray(x, np.float32))
    src = np.asarray(edge_index[0]).astype(np.int64)
    dst = np.asarray(edge_index[1]).astype(np.int64)
    batch = np.asarray(batch).astype(np.int64)

    deg = (1.0 + np.bincount(dst, minlength=N)).astype(np.float32)
    dis = (1.0 / np.sqrt(deg)).astype(np.float32)
    enorm = dis[src] * dis[dst]
    snorm = (dis * dis).astype(np.float32)

    counts = np.bincount(batch, minlength=G).astype(np.float32)
    recip = (1.0 / np.maximum(counts, 1.0)).astype(np.float32)

    # per-core edge lists sharded by dst, sorted by (half, local dst)
    per_core = []
    core_of = dst // NP
    for c in range(NCORES):
        m = core_of == c
        s, dl, w = src[m], dst[m] - c * NP, enorm[m]
        h = (s >= HALF).astype(np.int64)
        order = np.lexsort((dl, h))
        per_core.append((s[order], dl[order], w[order], h[order]))

    # shared static tile schedule: per (window, half), max tiles over cores
    nwin = cfg.nwin
    cnt = np.zeros((NCORES, nwin, 2), np.int64)
    bounds = []
    for c in range(NCORES):
        s, dl, w, h = per_core[c]
        nlo = int(np.searchsorted(h, 1))
        blo = np.searchsorted(dl[:nlo], np.arange(nwin + 1) * P)
        bhi = nlo + np.searchsorted(dl[nlo:], np.arange(nwin + 1) * P)
        bounds.append((blo, bhi))
        cnt[c, :, 0] = blo[1:] - blo[:-1]
        cnt[c, :, 1] = bhi[1:] - bhi[:-1]
    tiles_wh = np.max(-(-cnt // P), axis=0)      # [nwin, 2]
    T_lo = int(tiles_wh[:, 0].sum())
    T_hi = int(tiles_wh[:, 1].sum())
    T = T_lo + T_hi

    src_rel = np.zeros((NCORES, P, T), np.int16)
    selmeta = np.zeros((NCORES, P, 2 * T), np.float32)
    for c in range(NCORES):
        s, dl, w, h = per_core[c]
        blo, bhi = bounds[c]
        for half in (0, 1):
            t0 = 0 if half == 0 else T_lo
            bb = blo if half == 0 else bhi
            for wi in range(nwin):
                for j in range(int(tiles_wh[wi, half])):
                    a = bb[wi] + j * P
                    n = max(0, min(P, bb[wi + 1] - a))
                    t = t0 + j
                    if n > 0:
                        src_rel[c, :n, t] = (s[a:a + n] - half * HALF)
                        selmeta[c, :n, 2 * t] = (dl[a:a + n] - wi * P)
                        selmeta[c, :n, 2 * t + 1] = w[a:a + n]
                t0 += int(tiles_wh[wi, half])

    # int16 index stream for dma_gather: flat position i -> [i%16, i//16],
    # replicated across the 8 16-partition groups
    idx16 = np.zeros((NCORES, P, 8 * T), np.int16)
    for c in range(NCORES):
        flat = src_rel[c].T.reshape(-1)          # tile-major, then partition
        wrapped = flat.reshape(-1, 16).T         # [16, 8*T]
        idx16[c] = np.tile(wrapped, (8, 1))

    # per-core own-shard features in [node%128, tile*128+d] layout (layer-0
    # self-loop operand), zero-padded tail
    NT = nwin
    x_own = np.zeros((NCORES, P, NT * P), np.float32)
    snorm_pk = np.zeros((NCORES, P, NT), np.float32)
    batchf = np.full((NCORES, P, NT), -1.0, np.float32)
    for c in range(NCORES):
        xs = x[c * NP:(c + 1) * NP]
        pad = np.zeros((NT * P - NP, D), np.float32)
        x_own[c] = np.concatenate([xs, pad]).reshape(NT, P, D).transpose(
            1, 0, 2).reshape(P, NT * P)
        sn = np.concatenate([snorm[c * NP:(c + 1) * NP],
                             np.zeros(NT * P - NP, np.float32)])
        snorm_pk[c] = sn.reshape(NT, P).T
        ids = np.concatenate([batch[c * NP:(c + 1) * NP].astype(np.float32),
                              np.full(NT * P - NP, -1.0, np.float32)])
        batchf[c] = ids.reshape(NT, P).T

    iota = np.broadcast_to(np.arange(512, dtype=np.float32), (P, 512)).copy()
    Wpack = np.asarray(Ws, np.float32).transpose(1, 0, 2).reshape(D, cfg.L * D)
    gb = np.zeros((P, 2 * cfg.L), np.float32)
    for l in range(cfg.L):
        gb[:, 2 * l] = np.asarray(gammas[l], np.float32)
        gb[:, 2 * l + 1] = np.asarray(betas[l], np.float32)
    recip_pk = np.zeros((P, cfg.gblk), np.float32)
    for b in range(cfg.gblk):
        n = min(P, G - b * P)
        recip_pk[:n, b] = recip[b * P:b * P + n]

    shared = dict(x=x, wt=Wpack, gb=gb, iota=iota, recip=recip_pk)
    per_core_inputs = [dict(idx16=np.ascontiguousarray(idx16[c]),
                            selmeta=np.ascontiguousarray(selmeta[c]),
                            batchf=np.ascontiguousarray(batchf[c]),
                            xown=np.ascontiguousarray(x_own[c]),
                            snormpk=np.ascontiguousarray(snorm_pk[c]))
                       for c in range(NCORES)]
    sched = dict(tiles_wh=tiles_wh, T_lo=T_lo, T_hi=T_hi, T=T)
    return shared, per_core_inputs, sched


def build(cfg, sched, debug_dump=False):
    tiles_wh = sched["tiles_wh"]
    T_lo, T_hi, T = sched["T_lo"], sched["T_hi"], sched["T"]
    L, N, G, NP = cfg.L, cfg.N, cfg.G, cfg.NP

    nc = bacc.Bacc("TRN2", target_bir_lowering=False, debug=False,
                   num_devices=NCORES)
    dbg = {}
    if debug_dump:
        for nm, shape in [("zT", [P, cfg.nwin * P]),
                          ("hpre", [P, cfg.nblk * WBLK]),
                          ("stat", [P, 2]),
                          ("hnorm", [P, cfg.nblk * WBLK]),
                          ("hnew", [P, cfg.nwin * P]),
                          ("gdump", [P, GB_TILES * P])]:
            for l in range(L):
                dbg[f"{nm}{l}"] = nc.dram_tensor(
                    f"dbg_{nm}{l}", shape, F32, kind="ExternalOutput")

    x_e = nc.dram_tensor("x", [N, D], F32, kind="ExternalInput")
    wt_e = nc.dram_tensor("wt", [P, L * D], F32, kind="ExternalInput")
    gb_e = nc.dram_tensor("gb", [P, 2 * L], F32, kind="ExternalInput")
    iota_e = nc.dram_tensor("iota", [P, 512], F32, kind="ExternalInput")
    recip_e = nc.dram_tensor("recip", [P, cfg.gblk], F32, kind="ExternalInput")
    batchf_e = nc.dram_tensor("batchf", [P, cfg.nwin], F32, kind="ExternalInput")
    idx16_e = nc.dram_tensor("idx16", [P, 8 * T], I16, kind="ExternalInput")
    selmeta_e = nc.dram_tensor("selmeta", [P, 2 * T], F32, kind="ExternalInput")
    xown_e = nc.dram_tensor("xown", [P, cfg.nwin * P], F32, kind="ExternalInput")
    snorm_e = nc.dram_tensor("snormpk", [P, cfg.nwin], F32, kind="ExternalInput")
    out_e = nc.dram_tensor("out", [G, D], F32, kind="ExternalOutput")

    rg = [list(range(NCORES))]

    with tile.TileContext(nc) as tc:
        with tc.tile_pool(name="const", bufs=1) as cp, \
             tc.tile_pool(name="gpool", bufs=3) as gp, \
             tc.tile_pool(name="selp", bufs=8) as selp, \
             tc.tile_pool(name="mselp", bufs=3) as mselp, \
             tc.tile_pool(name="big", bufs=1) as bigp, \
             tc.tile_pool(name="scr", bufs=2) as scrp, \
             tc.tile_pool(name="small", bufs=4) as smp, \
             tc.tile_pool(name="pz", bufs=3, space="PSUM") as pzp, \
             tc.tile_pool(name="ph", bufs=2, space="PSUM") as php, \
             tc.tile_pool(name="pt", bufs=2, space="PSUM") as ptp, \
             tc.tile_pool(name="pg", bufs=1, space="PSUM") as pgp, \
             tc.tile_pool(name="dram", bufs=1, space="DRAM") as dp:

            # ---- constants into SBUF ----
            iota_sb = cp.tile([P, 512], F32)
            nc.sync.dma_start(out=iota_sb[:], in_=iota_e[:, :])
            wt_sb = cp.tile([P, L * D], F32)
            nc.sync.dma_start(out=wt_sb[:], in_=wt_e[:, :])
            gb_sb = cp.tile([P, 2 * L], F32)
            nc.sync.dma_start(out=gb_sb[:], in_=gb_e[:, :])
            recip_sb = cp.tile([P, cfg.gblk], F32)
            nc.sync.dma_start(out=recip_sb[:], in_=recip_e[:, :])
            batchf_sb = cp.tile([P, cfg.nwin], F32)
            nc.sync.dma_start(out=batchf_sb[:], in_=batchf_e[:, :])
            idx_sb = cp.tile([P, 8 * T], I16)
            nc.sync.dma_start(out=idx_sb[:], in_=idx16_e[:, :])
            meta_sb = cp.tile([P, 2 * T], F32)
            nc.sync.dma_start(out=meta_sb[:], in_=selmeta_e[:, :])
            xown_sb = cp.tile([P, cfg.nwin * P], F32)
            nc.sync.dma_start(out=xown_sb[:], in_=xown_e[:, :])
            snorm_sb = cp.tile([P, cfg.nwin], F32)
            nc.sync.dma_start(out=snorm_sb[:], in_=snorm_e[:, :])
            ident = cp.tile([P, P], F32)
            make_identity(nc, ident[:])
            zero_c = cp.tile([P, 1], F32)
            nc.vector.memset(zero_c[:], 0.0)
            eps_c = cp.tile([P, 1], F32)
            nc.vector.memset(eps_c[:], EPS)

            hnew = bigp.tile([P, cfg.nwin * P], F32)
            if cfg.rem:
                # matmuls read the full 128 partitions of the last node
                # tile; make the unwritten tail well-defined zeros (runs
                # once; per-layer transposes overwrite only valid rows)
                nc.vector.memset(hnew[:, cfg.nfull * P:], 0.0)

            tables = []
            for l in range(L - 1):
                tables.append(dp.tile([N, D], F32, addr_space="Shared",
                                      name=f"table{l}"))

            # per-(window,half) phase-local first tile index
            starts = np.zeros((cfg.nwin, 2), np.int64)
            t0 = 0
            for w in range(cfg.nwin):
                starts[w, 0] = t0
                t0 += int(tiles_wh[w][0])
            t0 = 0
            for w in range(cfg.nwin):
                starts[w, 1] = t0
                t0 += int(tiles_wh[w][1])

            for l in range(L):
                table_src = x_e if l == 0 else tables[l - 1]
                hprev = xown_sb if l == 0 else hnew

                zT = bigp.tile([P, cfg.nwin * P], F32, tag="zT")
                gather_tiles = [{}, {}]

                def ensure_gather(half, t_local, l=l, table_src=table_src,
                                  gather_tiles=gather_tiles):
                    """Issue the dma_gather covering phase-local tile t_local
                    of the given half; returns the gather SBUF tile."""
                    k = t_local // GB_TILES
                    cache = gather_tiles[half]
                    if k not in cache:
                        T_ph = T_lo if half == 0 else T_hi
                        cnt_t = min(GB_TILES, T_ph - k * GB_TILES)
                        g = gp.tile([P, GB_TILES * P], F32, tag="gath")
                        base = (0 if half == 0 else T_lo) + k * GB_TILES
                        tbl = table_src[:, :] if half == 0 \
                            else table_src[cfg.HALF:, :]
                        nc.gpsimd.dma_gather(
                            out_ap=g[:, :cnt_t * P].rearrange(
                                "p (t d) -> p t d", d=P),
                            in_ap=tbl,
                            idxs_ap=idx_sb[:, base * 8:(base + cnt_t) * 8],
                            num_idxs=cnt_t * P,
                            num_idxs_reg=cnt_t * P,
                            elem_size=D,
                            single_packet=False,
                        )
                        if debug_dump and half == 0 and k == 0:
                            nc.sync.dma_start(out=dbg[f"gdump{l}"][:, :],
                                              in_=g[:])
                        cache[k] = g
                    return cache[k]

                # --- lo phase: self-loop diagonal + lo-half edge tiles ---
                for w in range(cfg.nwin):
                    wlen = cfg.winlens[w]
                    nlo = int(tiles_wh[w][0])
                    pz = pzp.tile([P, P], F32, tag="pz")
                    dg = selp.tile([P, P], F32, tag="sel")
                    nc.vector.tensor_scalar(
                        out=dg[:, :wlen], in0=ident[:, :wlen],
                        scalar1=snorm_sb[:, w:w + 1], scalar2=None,
                        op0=OP.mult)
                    nc.tensor.matmul(out=pz[:, :wlen],
                                     lhsT=hprev[:, w * P:(w + 1) * P],
                                     rhs=dg[:, :wlen],
                                     start=True, stop=(nlo == 0))
                    for j in range(nlo):
                        t = int(starts[w, 0]) + j
                        g = ensure_gather(0, t)
                        slot = t % GB_TILES
                        sel = selp.tile([P, P], F32, tag="sel")
                        nc.vector.tensor_scalar(
                            out=sel[:, :wlen], in0=iota_sb[:, :wlen],
                            scalar1=meta_sb[:, 2 * t:2 * t + 1],
                            scalar2=meta_sb[:, 2 * t + 1:2 * t + 2],
                            op0=OP.is_equal, op1=OP.mult)
                        nc.tensor.matmul(out=pz[:, :wlen],
                                         lhsT=g[:, slot * P:(slot + 1) * P],
                                         rhs=sel[:, :wlen],
                                         start=False, stop=(j == nlo - 1))
                    nc.vector.tensor_copy(out=zT[:, w * P:w * P + wlen],
                                          in_=pz[:, :wlen])

                # --- hi phase: hi-half edge tiles, added into zT ---
                for w in range(cfg.nwin):
                    wlen = cfg.winlens[w]
                    nhi = int(tiles_wh[w][1])
                    if nhi == 0:
                        continue
                    pz = pzp.tile([P, P], F32, tag="pz")
                    for j in range(nhi):
                        t = int(starts[w, 1]) + j
                        g = ensure_gather(1, t)
                        slot = t % GB_TILES
                        tm = T_lo + t
                        sel = selp.tile([P, P], F32, tag="sel")
                        nc.vector.tensor_scalar(
                            out=sel[:, :wlen], in0=iota_sb[:, :wlen],
                            scalar1=meta_sb[:, 2 * tm:2 * tm + 1],
                            scalar2=meta_sb[:, 2 * tm + 1:2 * tm + 2],
                            op0=OP.is_equal, op1=OP.mult)
                        nc.tensor.matmul(out=pz[:, :wlen],
                                         lhsT=g[:, slot * P:(slot + 1) * P],
                                         rhs=sel[:, :wlen],
                                         start=(j == 0), stop=(j == nhi - 1))
                    nc.vector.tensor_tensor(out=zT[:, w * P:w * P + wlen],
                                            in0=zT[:, w * P:w * P + wlen],
                                            in1=pz[:, :wlen], op=OP.add)

                if debug_dump:
                    nc.sync.dma_start(out=dbg[f"zT{l}"][:, :], in_=zT[:])

                # ---- W matmul + BN stats ----
                hpre = bigp.tile([P, cfg.nblk * WBLK], F32, tag="hpre")
                sums = smp.tile([P, cfg.nblk], F32, tag="sums")
                sqs = smp.tile([P, cfg.nblk], F32, tag="sqs")
                for b in range(cfg.nblk):
                    blen = cfg.blens[b]
                    ph = php.tile([P, WBLK], F32, tag="ph")
                    nc.tensor.matmul(
                        out=ph[:, :blen],
                        lhsT=wt_sb[:, l * D:(l + 1) * D],
                        rhs=zT[:, b * WBLK:b * WBLK + blen],
                        start=True, stop=True)
                    nc.scalar.activation(
                        out=hpre[:, b * WBLK:b * WBLK + blen],
                        in_=ph[:, :blen], func=ACTF.Copy,
                        accum_out=sums[:, b:b + 1])
                    scr = scrp.tile([P, WBLK], F32, tag="scr")
                    nc.scalar.activation(
                        out=scr[:, :blen], in_=ph[:, :blen], func=ACTF.Square,
                        bias=zero_c[:, :1], accum_out=sqs[:, b:b + 1])

                ssum = smp.tile([P, 1], F32, tag="ssum")
                ssq = smp.tile([P, 1], F32, tag="ssq")
                nc.vector.reduce_sum(out=ssum[:], in_=sums[:],
                                     axis=mybir.AxisListType.X)
                nc.vector.reduce_sum(out=ssq[:], in_=sqs[:],
                                     axis=mybir.AxisListType.X)
                statpk = smp.tile([P, 2], F32, tag="statpk")
                nc.vector.tensor_copy(out=statpk[:, 0:1], in_=ssum[:])
                nc.vector.tensor_copy(out=statpk[:, 1:2], in_=ssq[:])
                stat_in = dp.tile([P, 2], F32, name=f"statin{l}")
                stat_out = dp.tile([P, 2], F32, addr_space="Shared",
                                   name=f"statout{l}")
                nc.sync.dma_start(out=stat_in[:], in_=statpk[:])
                nc.gpsimd.collective_compute(
                    "AllReduce", OP.add, replica_groups=rg,
                    ins=[stat_in[:].opt()], outs=[stat_out[:].opt()])
                statred = smp.tile([P, 2], F32, tag="statred")
                nc.sync.dma_start(out=statred[:], in_=stat_out[:])

                if debug_dump:
                    nc.sync.dma_start(out=dbg[f"hpre{l}"][:, :], in_=hpre[:])
                    nc.sync.dma_start(out=dbg[f"stat{l}"][:, :], in_=statred[:])

                mu = smp.tile([P, 1], F32, tag="mu")
                ex2 = smp.tile([P, 1], F32, tag="ex2")
                var = smp.tile([P, 1], F32, tag="var")
                std = smp.tile([P, 1], F32, tag="std")
                rsinv = smp.tile([P, 1], F32, tag="rsinv")
                s1 = smp.tile([P, 1], F32, tag="s1")
                s2 = smp.tile([P, 1], F32, tag="s2")
                inv_n = float(np.float32(1.0 / N))
                nc.vector.tensor_scalar(out=mu[:], in0=statred[:, 0:1],
                                        scalar1=inv_n, scalar2=None,
                                        op0=OP.mult)
                nc.vector.tensor_scalar(out=ex2[:], in0=statred[:, 1:2],
                                        scalar1=inv_n, scalar2=None,
                                        op0=OP.mult)
                nc.vector.scalar_tensor_tensor(
                    out=var[:], in0=mu[:], scalar=1.0, in1=mu[:],
                    op0=OP.bypass, op1=OP.mult)
                nc.vector.tensor_tensor(out=var[:], in0=ex2[:], in1=var[:],
                                        op=OP.subtract)
                nc.scalar.activation(out=std[:], in_=var[:], func=ACTF.Sqrt,
                                     bias=eps_c[:, :1])
                nc.vector.reciprocal(out=rsinv[:], in_=std[:])
                nc.vector.tensor_tensor(out=s1[:], in0=gb_sb[:, 2 * l:2 * l + 1],
                                        in1=rsinv[:], op=OP.mult)
                nc.vector.tensor_tensor(out=s2[:], in0=mu[:], in1=s1[:],
                                        op=OP.mult)
                nc.vector.tensor_tensor(out=s2[:],
                                        in0=gb_sb[:, 2 * l + 1:2 * l + 2],
                                        in1=s2[:], op=OP.subtract)

                # ---- normalize (+relu), transpose back to [node, D] ----
                hnorm = bigp.tile([P, cfg.nblk * WBLK], F32, tag="hnorm")
                for b in range(cfg.nblk):
                    blen = cfg.blens[b]
                    sl = slice(b * WBLK, b * WBLK + blen)
                    if l < L - 1:
                        nc.scalar.activation(out=hnorm[:, sl], in_=hpre[:, sl],
                                             func=ACTF.Relu, bias=s2[:, :1],
                                             scale=s1[:, :1])
                    else:
                        nc.vector.tensor_scalar(out=hnorm[:, sl],
                                                in0=hpre[:, sl],
                                                scalar1=s1[:, :1],
                                                scalar2=s2[:, :1],
                                                op0=OP.mult, op1=OP.add)
                if debug_dump:
                    nc.sync.dma_start(out=dbg[f"hnorm{l}"][:, :], in_=hnorm[:])
                for nt in range(cfg.nwin):
                    tl = cfg.winlens[nt]
                    pt = ptp.tile([P, P], F32, tag="pt")
                    nc.tensor.transpose(out=pt[:tl, :],
                                        in_=hnorm[:, nt * P:nt * P + tl],
                                        identity=ident[:])
                    nc.vector.tensor_copy(out=hnew[:tl, nt * P:(nt + 1) * P],
                                          in_=pt[:tl, :])
                if debug_dump:
                    nc.sync.dma_start(out=dbg[f"hnew{l}"][:, :], in_=hnew[:])

                if l < L - 1:
                    # ---- write shard + AllGather the next-layer table ----
                    part = dp.tile([NP, D], F32, name=f"part{l}")
                    if cfg.nfull:
                        nc.sync.dma_start(
                            out=part[:cfg.nfull * P, :].rearrange(
                                "(nt p) d -> p nt d", p=P),
                            in_=hnew[:, :cfg.nfull * P].rearrange(
                                "p (nt d) -> p nt d", d=D))
                    if cfg.rem:
                        nc.sync.dma_start(
                            out=part[cfg.nfull * P:, :],
                            in_=hnew[:cfg.rem, cfg.nfull * P:(cfg.nfull + 1) * P])
                    nc.gpsimd.collective_compute(
                        "AllGather", OP.bypass, replica_groups=rg,
                        ins=[part[:].opt()], outs=[tables[l][:].opt()])

            # ---- global mean pool ----
            pgps = pgp.tile([P, G], F32)
            for nt in range(cfg.nwin):
                msel = mselp.tile([P, G], F32, tag="msel")
                nc.vector.tensor_scalar(out=msel[:], in0=iota_sb[:, :G],
                                        scalar1=batchf_sb[:, nt:nt + 1],
                                        scalar2=None, op0=OP.is_equal)
                nc.tensor.matmul(out=pgps[:], lhsT=hnew[:, nt * P:(nt + 1) * P],
                                 rhs=msel[:], start=(nt == 0),
                                 stop=(nt == cfg.nwin - 1))
            poolsb = cp.tile([P, G], F32)
            nc.vector.tensor_copy(out=poolsb[:], in_=pgps[:])
            pool_in = dp.tile([P, G], F32, name="poolin")
            pool_out = dp.tile([P, G], F32, addr_space="Shared", name="poolout")
            nc.sync.dma_start(out=pool_in[:], in_=poolsb[:])
            nc.gpsimd.collective_compute(
                "AllReduce", OP.add, replica_groups=rg,
                ins=[pool_in[:].opt()], outs=[pool_out[:].opt()])
            poolred = cp.tile([P, G], F32)
            nc.sync.dma_start(out=poolred[:], in_=pool_out[:])
            outsb = cp.tile([P, cfg.gblk * D], F32)
            for b in range(cfg.gblk):
                gl = min(P, G - b * P)
                pt = ptp.tile([P, P], F32, tag="pt")
                nc.tensor.transpose(out=pt[:gl, :],
                                    in_=poolred[:, b * P:b * P + gl],
                                    identity=ident[:])
                nc.vector.tensor_scalar(out=outsb[:gl, b * D:(b + 1) * D],
                                        in0=pt[:gl, :],
                                        scalar1=recip_sb[:gl, b:b + 1],
                                        scalar2=None, op0=OP.mult)
            if cfg.gblk == 1:
                nc.sync.dma_start(out=out_e[:, :], in_=outsb[:G, :D])
            else:
                nc.sync.dma_start(
                    out=out_e[:, :].rearrange("(b g) d -> g b d", g=P),
                    in_=outsb[:, :].rearrange("g (b d) -> g b d", d=D))
    nc.compile()
    return nc


_CACHE = {}


def _get_compiled(cfg, sched_key, sched, debug_dump=False):
    key = (cfg.N, cfg.E, cfg.G, cfg.L, sched_key, debug_dump)
    if key not in _CACHE:
        _CACHE[key] = build(cfg, sched, debug_dump=debug_dump)
    return _CACHE[key]


def run(cfg, inputs, trace=False, debug_dump=False):
    shared, per_core, sched = host_preprocess(cfg, **inputs)
    sched_key = (sched["T_lo"], sched["T_hi"],
                 tuple(map(tuple, sched["tiles_wh"])))
    nc = _get_compiled(cfg, sched_key, sched, debug_dump=debug_dump)
    in_maps = [dict(shared, **pc) for pc in per_core]
    res = bass_utils.run_bass_kernel_spmd(
        nc, in_maps, core_ids=list(range(NCORES)), trace=trace)
    out = res.results[0]["out"]
    return out, res


def build_null(cfg, sched):
    """Same external I/O signature as build(), trivial compute — used to
    subtract host/RPC/dispatch overhead from wall-clock timing."""
    T = sched["T"]
    L, N, G = cfg.L, cfg.N, cfg.G
    nc = bacc.Bacc("TRN2", target_bir_lowering=False, debug=False,
                   num_devices=NCORES)
    nc.dram_tensor("x", [N, D], F32, kind="ExternalInput")
    nc.dram_tensor("wt", [P, L * D], F32, kind="ExternalInput")
    nc.dram_tensor("gb", [P, 2 * L], F32, kind="ExternalInput")
    iota_e = nc.dram_tensor("iota", [P, 512], F32, kind="ExternalInput")
    nc.dram_tensor("recip", [P, cfg.gblk], F32, kind="ExternalInput")
    nc.dram_tensor("batchf", [P, cfg.nwin], F32, kind="ExternalInput")
    nc.dram_tensor("idx16", [P, 8 * T], I16, kind="ExternalInput")
    nc.dram_tensor("selmeta", [P, 2 * T], F32, kind="ExternalInput")
    nc.dram_tensor("xown", [P, cfg.nwin * P], F32, kind="ExternalInput")
    nc.dram_tensor("snormpk", [P, cfg.nwin], F32, kind="ExternalInput")
    out_e = nc.dram_tensor("out", [G, D], F32, kind="ExternalOutput")
    with tile.TileContext(nc) as tc:
        with tc.tile_pool(name="sb", bufs=1) as sb:
            t = sb.tile([P, D], F32)
            nc.sync.dma_start(out=t[:], in_=iota_e[:, :D])
            for b in range(-(-G // P)):
                gl = min(P, G - b * P)
                nc.sync.dma_start(out=out_e[b * P:b * P + gl, :],
                                  in_=t[:gl, :])
    nc.compile()
    return nc


def time_pjrt(nc, in_maps, iters=8, warmup=2):
    """Wall-clock repeated executions of the compiled program with
    device-resident inputs (mirrors bass2jax.run_bass_via_pjrt)."""
    import time
    import jax
    from jax.sharding import Mesh, PartitionSpec, NamedSharding
    from jax.experimental.shard_map import shard_map
    from concourse import bass2jax
    from concourse import mybir as mb

    bass2jax.install_neuronx_cc_hook()
    partition_name = (nc.partition_id_tensor.name
                      if nc.partition_id_tensor else None)
    in_names, out_names, out_avals, zero_outs = [], [], [], []
    for alloc in nc.m.functions[0].allocations:
        if not isinstance(alloc, mb.MemoryLocationSet):
            continue
        name = alloc.memorylocations[0].name
        if alloc.kind == "ExternalInput":
            if name != partition_name:
                in_names.append(name)
        elif alloc.kind == "ExternalOutput":
            out_names.append(name)
            shape = tuple(alloc.tensor_shape)
            dtype = mb.dt.np(alloc.dtype)
            out_avals.append(jax.core.ShapedArray(shape, dtype))
            zero_outs.append(np.zeros(shape, dtype))
    n_params = len(in_names)
    in_names = in_names + out_names
    if partition_name is not None:
        in_names.append(partition_name)

    def _body(*args):
        operands = list(args)
        if partition_name is not None:
            operands.append(bass2jax.partition_id_tensor())
        outs = bass2jax._bass_exec_p.bind(
            *operands, out_avals=tuple(out_avals), in_names=tuple(in_names),
            out_names=tuple(out_names), lowering_input_output_aliases=(),
            sim_require_finite=True, sim_require_nnan=True, nc=nc)
        return tuple(outs)

    devices = jax.devices()[:NCORES]
    mesh = Mesh(np.asarray(devices), ("core",))
    spec = PartitionSpec("core")
    in_specs = (spec,) * (n_params + len(out_names))
    out_specs = (spec,) * len(out_names)
    fn = jax.jit(shard_map(_body, mesh=mesh, in_specs=in_specs,
                           out_specs=out_specs, check_rep=False),
                 keep_unused=True)
    sharding = NamedSharding(mesh, spec)
    concat_in = [
        jax.device_put(np.concatenate(
            [np.asarray(in_maps[c][in_names[i]]) for c in range(NCORES)],
            axis=0), sharding)
        for i in range(n_params)
    ]
    concat_zeros = [
        jax.device_put(np.zeros((NCORES * z.shape[0], *z.shape[1:]), z.dtype),
                       sharding)
        for z in zero_outs
    ]
    jax.block_until_ready(concat_in)
    for _ in range(warmup):
        jax.block_until_ready(fn(*concat_in, *concat_zeros))
    times = []
    for _ in range(iters):
        t0 = time.perf_counter()
        jax.block_until_ready(fn(*concat_in, *concat_zeros))
        times.append(time.perf_counter() - t0)
    return times


def measure(cfg, inputs, iters=8):
    """Returns (kernel_walls, null_walls) in seconds."""
    shared, per_core, sched = host_preprocess(cfg, **inputs)
    sched_key = (sched["T_lo"], sched["T_hi"],
                 tuple(map(tuple, sched["tiles_wh"])))
    nc = _get_compiled(cfg, sched_key, sched)
    in_maps = [dict(shared, **pc) for pc in per_core]
    kw = time_pjrt(nc, in_maps, iters=iters)
    key = ("null", cfg.N, cfg.E, cfg.G, sched_key)
    if key not in _CACHE:
        _CACHE[key] = build_null(cfg, sched)
    nw = time_pjrt(_CACHE[key], in_maps, iters=iters)
    return kw, nw


def kernel(x, edge_index, batch, Ws, bs, gammas, betas):
    cfg = Cfg(N=50000, E=625000, G=256, L=3)
    out, _ = run(cfg, dict(x=x, edge_index=edge_index, batch=batch, Ws=Ws,
                           bs=bs, gammas=gammas, betas=betas))
    return out


# revision 25
# speedup vs baseline: 1.5723x; 1.5723x over previous
"""GCN encoder (3-layer GCNConv + BatchNorm + ReLU + global mean pool) on 8
Trainium2 NeuronCores.

Strategy (graph/data parallel, edges sharded by destination):
  - Nodes are split into 8 contiguous shards (one per core). Each core owns
    all edges whose destination lands in its shard.
  - The layer is computed aggregate-first (mathematically identical to the
    reference's transform-first order since GCNConv is linear):
        z[v]  = sum_{e: dst=v} w_e * h[src_e] + snorm_v * h[v]
        hpre  = W.T @ z                          (kept transposed: [D, nodes])
        h_out = relu(gamma * (hpre - mu) / sqrt(var+eps) + beta)
  - The gather h[src_e] uses dma_gather (int16 indices, 2048 rows per
    instruction) from a replicated full node table in HBM. Since int16
    limits the index range, edges are split into two phases by source half
    (lo: src < N/2, hi: src >= N/2 gathered from an offset table view).
  - Per 128-edge tile, a selection matrix Sel[e, dst_local] = w_e is built
    on-chip with one tensor_scalar(is_equal, mult) against an iota constant,
    and the scatter-add becomes a PE matmul G.T @ Sel accumulated in PSUM
    over a 128-destination window. Self-loops are a diagonal-matrix matmul
    against the previous layer's activations already in SBUF.
  - BatchNorm statistics are free-axis reductions in the transposed layout;
    partials are combined with a [128,2] AllReduce. After normalization the
    result is transposed back (PE transpose) and AllGathered into the next
    layer's node table.
  - Mean pooling reuses the selection-matmul trick against the sorted graph
    ids, followed by a [128,256] AllReduce and division by counts.
"""

import sys

sys.path.insert(0, "/opt/trn_rl_repo")

import numpy as np

import concourse.bass as bass
import concourse.tile as tile
from concourse import bacc, mybir
from concourse import bass_utils
from concourse.masks import make_identity

F32 = mybir.dt.float32
I16 = mybir.dt.int16
OP = mybir.AluOpType
ACTF = mybir.ActivationFunctionType

NCORES = 8
D = 128
P = 128
GB_TILES = 16     # 128-edge tiles per dma_gather
WBLK = 512        # node columns per W-matmul / BN block
EPS = 1e-5


class Cfg:
    def __init__(self, N, E, G, L=3):
        assert N % NCORES == 0
        self.N, self.E, self.G, self.L = N, E, G, L
        self.NP = N // NCORES                    # nodes per core
        self.nwin = -(-self.NP // P)             # 128-dst windows per core
        assert self.nwin >= 2
        # each shard splits into half A (first NFA full node tiles) and
        # half B; the two AllGathers pipeline against the next layer's
        # phase-A gathers
        self.NFA = self.nwin // 2
        self.HA = self.NFA * P
        self.HB = self.NP - self.HA
        assert NCORES * max(self.HA, self.HB) < 32768
        self.winlens = [min(P, self.NP - w * P) for w in range(self.nwin)]
        self.nblk = -(-self.NP // WBLK)          # 512-node BN/W blocks
        self.blens = [min(WBLK, self.NP - b * WBLK) for b in range(self.nblk)]
        self.nfull = self.NP // P                # full 128-node tiles
        self.rem = self.NP - self.nfull * P
        self.gblk = -(-G // P)                   # 128-graph output tiles
        assert self.gblk * P == G or G <= P


def host_preprocess(cfg, x, edge_index, batch, Ws, bs, gammas, betas):
    """Shard + sort edges, build per-core packed metadata arrays."""
    N, G = cfg.N, cfg.G
    NP = cfg.NP
    x = np.ascontiguousarray(np.asarray(x, np.float32))
    src = np.asarray(edge_index[0]).astype(np.int64)
    dst = np.asarray(edge_index[1]).astype(np.int64)
    batch = np.asarray(batch).astype(np.int64)

    deg = (1.0 + np.bincount(dst, minlength=N)).astype(np.float32)
    dis = (1.0 / np.sqrt(deg)).astype(np.float32)
    enorm = dis[src] * dis[dst]
    snorm = (dis * dis).astype(np.float32)

    counts = np.bincount(batch, minlength=G).astype(np.float32)
    recip = (1.0 / np.maximum(counts, 1.0)).astype(np.float32)

    # per-core edge lists sharded by dst, sorted by (half, local dst);
    # the gather table is stored in AllGather order: half A = concat of all
    # cores' first HA rows, half B = concat of the rest
    per_core = []
    core_of = dst // NP
    for c in range(NCORES):
        m = core_of == c
        s, dl, w = src[m], dst[m] - c * NP, enorm[m]
        sc = s // NP
        sl = s - sc * NP
        h = (sl >= cfg.HA).astype(np.int64)
        rel = np.where(h == 0, sc * cfg.HA + sl, sc * cfg.HB + (sl - cfg.HA))
        order = np.lexsort((dl, h))
        per_core.append((rel[order], dl[order], w[order], h[order]))

    # shared static tile schedule: per (window, half), max tiles over cores
    nwin = cfg.nwin
    cnt = np.zeros((NCORES, nwin, 2), np.int64)
    bounds = []
    for c in range(NCORES):
        s, dl, w, h = per_core[c]
        nlo = int(np.searchsorted(h, 1))
        blo = np.searchsorted(dl[:nlo], np.arange(nwin + 1) * P)
        bhi = nlo + np.searchsorted(dl[nlo:], np.arange(nwin + 1) * P)
        bounds.append((blo, bhi))
        cnt[c, :, 0] = blo[1:] - blo[:-1]
        cnt[c, :, 1] = bhi[1:] - bhi[:-1]
    tiles_wh = np.max(-(-cnt // P), axis=0)      # [nwin, 2]
    T_lo = int(tiles_wh[:, 0].sum())
    T_hi = int(tiles_wh[:, 1].sum())
    T = T_lo + T_hi

    src_rel = np.zeros((NCORES, P, T), np.int16)
    selmeta = np.zeros((NCORES, P, 2 * T), np.float32)
    for c in range(NCORES):
        s, dl, w, h = per_core[c]
        blo, bhi = bounds[c]
        for half in (0, 1):
            t0 = 0 if half == 0 else T_lo
            bb = blo if half == 0 else bhi
            for wi in range(nwin):
                for j in range(int(tiles_wh[wi, half])):
                    a = bb[wi] + j * P
                    n = max(0, min(P, bb[wi + 1] - a))
                    t = t0 + j
                    if n > 0:
                        src_rel[c, :n, t] = s[a:a + n]
                        selmeta[c, :n, 2 * t] = (dl[a:a + n] - wi * P)
                        selmeta[c, :n, 2 * t + 1] = w[a:a + n]
                t0 += int(tiles_wh[wi, half])

    # int16 index stream for dma_gather: flat position i -> [i%16, i//16],
    # replicated across the 8 16-partition groups
    idx16 = np.zeros((NCORES, P, 8 * T), np.int16)
    for c in range(NCORES):
        flat = src_rel[c].T.reshape(-1)          # tile-major, then partition
        wrapped = flat.reshape(-1, 16).T         # [16, 8*T]
        idx16[c] = np.tile(wrapped, (8, 1))

    # per-core own-shard features in [node%128, tile*128+d] layout (layer-0
    # self-loop operand), zero-padded tail
    NT = nwin
    x_own = np.zeros((NCORES, P, NT * P), np.float32)
    snorm_pk = np.zeros((NCORES, P, NT), np.float32)
    batchf = np.full((NCORES, P, NT), -1.0, np.float32)
    for c in range(NCORES):
        xs = x[c * NP:(c + 1) * NP]
        pad = np.zeros((NT * P - NP, D), np.float32)
        x_own[c] = np.concatenate([xs, pad]).reshape(NT, P, D).transpose(
            1, 0, 2).reshape(P, NT * P)
        sn = np.concatenate([snorm[c * NP:(c + 1) * NP],
                             np.zeros(NT * P - NP, np.float32)])
        snorm_pk[c] = sn.reshape(NT, P).T
        ids = np.concatenate([batch[c * NP:(c + 1) * NP].astype(np.float32),
                              np.full(NT * P - NP, -1.0, np.float32)])
        batchf[c] = ids.reshape(NT, P).T

    iota = np.broadcast_to(np.arange(512, dtype=np.float32), (P, 512)).copy()
    Wpack = np.asarray(Ws, np.float32).transpose(1, 0, 2).reshape(D, cfg.L * D)
    gb = np.zeros((P, 2 * cfg.L), np.float32)
    for l in range(cfg.L):
        gb[:, 2 * l] = np.asarray(gammas[l], np.float32)
        gb[:, 2 * l + 1] = np.asarray(betas[l], np.float32)
    recip_pk = np.zeros((P, cfg.gblk), np.float32)
    for b in range(cfg.gblk):
        n = min(P, G - b * P)
        recip_pk[:n, b] = recip[b * P:b * P + n]

    xa = np.concatenate([x[c * NP:c * NP + cfg.HA] for c in range(NCORES)])
    xb = np.concatenate([x[c * NP + cfg.HA:(c + 1) * NP]
                         for c in range(NCORES)])
    shared = dict(xa=np.ascontiguousarray(xa), xb=np.ascontiguousarray(xb),
                  wt=Wpack, gb=gb, iota=iota, recip=recip_pk)
    per_core_inputs = [dict(idx16=np.ascontiguousarray(idx16[c]),
                            selmeta=np.ascontiguousarray(selmeta[c]),
                            batchf=np.ascontiguousarray(batchf[c]),
                            xown=np.ascontiguousarray(x_own[c]),
                            snormpk=np.ascontiguousarray(snorm_pk[c]))
                       for c in range(NCORES)]
    sched = dict(tiles_wh=tiles_wh, T_lo=T_lo, T_hi=T_hi, T=T)
    return shared, per_core_inputs, sched


def build(cfg, sched, debug_dump=False):
    tiles_wh = sched["tiles_wh"]
    T_lo, T_hi, T = sched["T_lo"], sched["T_hi"], sched["T"]
    L, N, G, NP = cfg.L, cfg.N, cfg.G, cfg.NP

    nc = bacc.Bacc("TRN2", target_bir_lowering=False, debug=False,
                   num_devices=NCORES)
    dbg = {}
    if debug_dump:
        for nm, shape in [("zT", [P, cfg.nwin * P]),
                          ("hpre", [P, cfg.nblk * WBLK]),
                          ("stat", [P, 2]),
                          ("hnorm", [P, cfg.nblk * WBLK]),
                          ("hnew", [P, cfg.nwin * P]),
                          ("gdump", [P, GB_TILES * P])]:
            for l in range(L):
                dbg[f"{nm}{l}"] = nc.dram_tensor(
                    f"dbg_{nm}{l}", shape, F32, kind="ExternalOutput")

    xa_e = nc.dram_tensor("xa", [NCORES * cfg.HA, D], F32,
                          kind="ExternalInput")
    xb_e = nc.dram_tensor("xb", [NCORES * cfg.HB, D], F32,
                          kind="ExternalInput")
    wt_e = nc.dram_tensor("wt", [P, L * D], F32, kind="ExternalInput")
    gb_e = nc.dram_tensor("gb", [P, 2 * L], F32, kind="ExternalInput")
    iota_e = nc.dram_tensor("iota", [P, 512], F32, kind="ExternalInput")
    recip_e = nc.dram_tensor("recip", [P, cfg.gblk], F32, kind="ExternalInput")
    batchf_e = nc.dram_tensor("batchf", [P, cfg.nwin], F32, kind="ExternalInput")
    idx16_e = nc.dram_tensor("idx16", [P, 8 * T], I16, kind="ExternalInput")
    selmeta_e = nc.dram_tensor("selmeta", [P, 2 * T], F32, kind="ExternalInput")
    xown_e = nc.dram_tensor("xown", [P, cfg.nwin * P], F32, kind="ExternalInput")
    snorm_e = nc.dram_tensor("snormpk", [P, cfg.nwin], F32, kind="ExternalInput")
    out_e = nc.dram_tensor("out", [G, D], F32, kind="ExternalOutput")

    rg = [list(range(NCORES))]

    with tile.TileContext(nc) as tc:
        with tc.tile_pool(name="const", bufs=1) as cp, \
             tc.tile_pool(name="gpool", bufs=3) as gp, \
             tc.tile_pool(name="selp", bufs=8) as selp, \
             tc.tile_pool(name="mselp", bufs=3) as mselp, \
             tc.tile_pool(name="big", bufs=1) as bigp, \
             tc.tile_pool(name="scr", bufs=2) as scrp, \
             tc.tile_pool(name="small", bufs=4) as smp, \
             tc.tile_pool(name="pz", bufs=3, space="PSUM") as pzp, \
             tc.tile_pool(name="ph", bufs=2, space="PSUM") as php, \
             tc.tile_pool(name="pt", bufs=2, space="PSUM") as ptp, \
             tc.tile_pool(name="pg", bufs=1, space="PSUM") as pgp, \
             tc.tile_pool(name="dram", bufs=1, space="DRAM") as dp:

            # ---- constants into SBUF ----
            iota_sb = cp.tile([P, 512], F32)
            nc.sync.dma_start(out=iota_sb[:], in_=iota_e[:, :])
            wt_sb = cp.tile([P, L * D], F32)
            nc.sync.dma_start(out=wt_sb[:], in_=wt_e[:, :])
            gb_sb = cp.tile([P, 2 * L], F32)
            nc.sync.dma_start(out=gb_sb[:], in_=gb_e[:, :])
            recip_sb = cp.tile([P, cfg.gblk], F32)
            nc.sync.dma_start(out=recip_sb[:], in_=recip_e[:, :])
            batchf_sb = cp.tile([P, cfg.nwin], F32)
            nc.sync.dma_start(out=batchf_sb[:], in_=batchf_e[:, :])
            idx_sb = cp.tile([P, 8 * T], I16)
            nc.sync.dma_start(out=idx_sb[:], in_=idx16_e[:, :])
            meta_sb = cp.tile([P, 2 * T], F32)
            nc.sync.dma_start(out=meta_sb[:], in_=selmeta_e[:, :])
            xown_sb = cp.tile([P, cfg.nwin * P], F32)
            nc.sync.dma_start(out=xown_sb[:], in_=xown_e[:, :])
            snorm_sb = cp.tile([P, cfg.nwin], F32)
            nc.sync.dma_start(out=snorm_sb[:], in_=snorm_e[:, :])
            ident = cp.tile([P, P], F32)
            make_identity(nc, ident[:])
            zero_c = cp.tile([P, 1], F32)
            nc.vector.memset(zero_c[:], 0.0)
            eps_c = cp.tile([P, 1], F32)
            nc.vector.memset(eps_c[:], EPS)

            hnew = bigp.tile([P, cfg.nwin * P], F32)
            if cfg.rem:
                # matmuls read the full 128 partitions of the last node
                # tile; make the unwritten tail well-defined zeros (runs
                # once; per-layer transposes overwrite only valid rows)
                nc.vector.memset(hnew[:, cfg.nfull * P:], 0.0)

            tabA, tabB = [], []
            for l in range(L - 1):
                tabA.append(dp.tile([NCORES * cfg.HA, D], F32,
                                    addr_space="Shared", name=f"tabA{l}"))
                tabB.append(dp.tile([NCORES * cfg.HB, D], F32,
                                    addr_space="Shared", name=f"tabB{l}"))

            # per-(window,half) phase-local first tile index
            starts = np.zeros((cfg.nwin, 2), np.int64)
            t0 = 0
            for w in range(cfg.nwin):
                starts[w, 0] = t0
                t0 += int(tiles_wh[w][0])
            t0 = 0
            for w in range(cfg.nwin):
                starts[w, 1] = t0
                t0 += int(tiles_wh[w][1])

            for l in range(L):
                srcA = xa_e if l == 0 else tabA[l - 1]
                srcB = xb_e if l == 0 else tabB[l - 1]
                hprev = xown_sb if l == 0 else hnew

                zT = bigp.tile([P, cfg.nwin * P], F32, tag="zT")
                gather_tiles = [{}, {}]

                def ensure_gather(half, t_local, l=l, srcA=srcA,
                                  srcB=srcB, gather_tiles=gather_tiles):
                    """Issue the dma_gather covering phase-local tile t_local
                    of the given half; returns the gather SBUF tile."""
                    k = t_local // GB_TILES
                    cache = gather_tiles[half]
                    if k not in cache:
                        T_ph = T_lo if half == 0 else T_hi
                        cnt_t = min(GB_TILES, T_ph - k * GB_TILES)
                        g = gp.tile([P, GB_TILES * P], F32, tag="gath")
                        base = (0 if half == 0 else T_lo) + k * GB_TILES
                        tbl = srcA[:, :] if half == 0 else srcB[:, :]
                        nc.gpsimd.dma_gather(
                            out_ap=g[:, :cnt_t * P].rearrange(
                                "p (t d) -> p t d", d=P),
                            in_ap=tbl,
                            idxs_ap=idx_sb[:, base * 8:(base + cnt_t) * 8],
                            num_idxs=cnt_t * P,
                            num_idxs_reg=cnt_t * P,
                            elem_size=D,
                            single_packet=False,
                        )
                        if debug_dump and half == 0 and k == 0:
                            nc.sync.dma_start(out=dbg[f"gdump{l}"][:, :],
                                              in_=g[:])
                        cache[k] = g
                    return cache[k]

                # --- lo phase: self-loop diagonal + lo-half edge tiles ---
                for w in range(cfg.nwin):
                    wlen = cfg.winlens[w]
                    nlo = int(tiles_wh[w][0])
                    pz = pzp.tile([P, P], F32, tag="pz")
                    dg = selp.tile([P, P], F32, tag="sel")
                    nc.vector.tensor_scalar(
                        out=dg[:, :wlen], in0=ident[:, :wlen],
                        scalar1=snorm_sb[:, w:w + 1], scalar2=None,
                        op0=OP.mult)
                    nc.tensor.matmul(out=pz[:, :wlen],
                                     lhsT=hprev[:, w * P:(w + 1) * P],
                                     rhs=dg[:, :wlen],
                                     start=True, stop=(nlo == 0))
                    for j in range(nlo):
                        t = int(starts[w, 0]) + j
                        g = ensure_gather(0, t)
                        slot = t % GB_TILES
                        sel = selp.tile([P, P], F32, tag="sel")
                        nc.vector.tensor_scalar(
                            out=sel[:, :wlen], in0=iota_sb[:, :wlen],
                            scalar1=meta_sb[:, 2 * t:2 * t + 1],
                            scalar2=meta_sb[:, 2 * t + 1:2 * t + 2],
                            op0=OP.is_equal, op1=OP.mult)
                        nc.tensor.matmul(out=pz[:, :wlen],
                                         lhsT=g[:, slot * P:(slot + 1) * P],
                                         rhs=sel[:, :wlen],
                                         start=False, stop=(j == nlo - 1))
                    nc.vector.tensor_copy(out=zT[:, w * P:w * P + wlen],
                                          in_=pz[:, :wlen])

                # --- hi phase: hi-half edge tiles, added into zT ---
                for w in range(cfg.nwin):
                    wlen = cfg.winlens[w]
                    nhi = int(tiles_wh[w][1])
                    if nhi == 0:
                        continue
                    pz = pzp.tile([P, P], F32, tag="pz")
                    for j in range(nhi):
                        t = int(starts[w, 1]) + j
                        g = ensure_gather(1, t)
                        slot = t % GB_TILES
                        tm = T_lo + t
                        sel = selp.tile([P, P], F32, tag="sel")
                        nc.vector.tensor_scalar(
                            out=sel[:, :wlen], in0=iota_sb[:, :wlen],
                            scalar1=meta_sb[:, 2 * tm:2 * tm + 1],
                            scalar2=meta_sb[:, 2 * tm + 1:2 * tm + 2],
                            op0=OP.is_equal, op1=OP.mult)
                        nc.tensor.matmul(out=pz[:, :wlen],
                                         lhsT=g[:, slot * P:(slot + 1) * P],
                                         rhs=sel[:, :wlen],
                                         start=(j == 0), stop=(j == nhi - 1))
                    nc.vector.tensor_tensor(out=zT[:, w * P:w * P + wlen],
                                            in0=zT[:, w * P:w * P + wlen],
                                            in1=pz[:, :wlen], op=OP.add)

                if debug_dump:
                    nc.sync.dma_start(out=dbg[f"zT{l}"][:, :], in_=zT[:])

                # ---- W matmul + BN stats ----
                hpre = bigp.tile([P, cfg.nblk * WBLK], F32, tag="hpre")
                sums = smp.tile([P, cfg.nblk], F32, tag="sums")
                sqs = smp.tile([P, cfg.nblk], F32, tag="sqs")
                for b in range(cfg.nblk):
                    blen = cfg.blens[b]
                    ph = php.tile([P, WBLK], F32, tag="ph")
                    nc.tensor.matmul(
                        out=ph[:, :blen],
                        lhsT=wt_sb[:, l * D:(l + 1) * D],
                        rhs=zT[:, b * WBLK:b * WBLK + blen],
                        start=True, stop=True)
                    nc.scalar.activation(
                        out=hpre[:, b * WBLK:b * WBLK + blen],
                        in_=ph[:, :blen], func=ACTF.Copy,
                        accum_out=sums[:, b:b + 1])
                    scr = scrp.tile([P, WBLK], F32, tag="scr")
                    nc.scalar.activation(
                        out=scr[:, :blen], in_=ph[:, :blen], func=ACTF.Square,
                        bias=zero_c[:, :1], accum_out=sqs[:, b:b + 1])

                ssum = smp.tile([P, 1], F32, tag="ssum")
                ssq = smp.tile([P, 1], F32, tag="ssq")
                nc.vector.reduce_sum(out=ssum[:], in_=sums[:],
                                     axis=mybir.AxisListType.X)
                nc.vector.reduce_sum(out=ssq[:], in_=sqs[:],
                                     axis=mybir.AxisListType.X)
                statpk = smp.tile([P, 2], F32, tag="statpk")
                nc.vector.tensor_copy(out=statpk[:, 0:1], in_=ssum[:])
                nc.vector.tensor_copy(out=statpk[:, 1:2], in_=ssq[:])
                stat_in = dp.tile([P, 2], F32, name=f"statin{l}")
                stat_out = dp.tile([P, 2], F32, addr_space="Shared",
                                   name=f"statout{l}")
                nc.sync.dma_start(out=stat_in[:], in_=statpk[:])
                nc.gpsimd.collective_compute(
                    "AllReduce", OP.add, replica_groups=rg,
                    ins=[stat_in[:].opt()], outs=[stat_out[:].opt()])
                statred = smp.tile([P, 2], F32, tag="statred")
                nc.sync.dma_start(out=statred[:], in_=stat_out[:])

                if debug_dump:
                    nc.sync.dma_start(out=dbg[f"hpre{l}"][:, :], in_=hpre[:])
                    nc.sync.dma_start(out=dbg[f"stat{l}"][:, :], in_=statred[:])

                mu = smp.tile([P, 1], F32, tag="mu")
                ex2 = smp.tile([P, 1], F32, tag="ex2")
                var = smp.tile([P, 1], F32, tag="var")
                std = smp.tile([P, 1], F32, tag="std")
                rsinv = smp.tile([P, 1], F32, tag="rsinv")
                s1 = smp.tile([P, 1], F32, tag="s1")
                s2 = smp.tile([P, 1], F32, tag="s2")
                inv_n = float(np.float32(1.0 / N))
                nc.vector.tensor_scalar(out=mu[:], in0=statred[:, 0:1],
                                        scalar1=inv_n, scalar2=None,
                                        op0=OP.mult)
                nc.vector.tensor_scalar(out=ex2[:], in0=statred[:, 1:2],
                                        scalar1=inv_n, scalar2=None,
                                        op0=OP.mult)
                nc.vector.scalar_tensor_tensor(
                    out=var[:], in0=mu[:], scalar=1.0, in1=mu[:],
                    op0=OP.bypass, op1=OP.mult)
                nc.vector.tensor_tensor(out=var[:], in0=ex2[:], in1=var[:],
                                        op=OP.subtract)
                nc.scalar.activation(out=std[:], in_=var[:], func=ACTF.Sqrt,
                                     bias=eps_c[:, :1])
                nc.vector.reciprocal(out=rsinv[:], in_=std[:])
                nc.vector.tensor_tensor(out=s1[:], in0=gb_sb[:, 2 * l:2 * l + 1],
                                        in1=rsinv[:], op=OP.mult)
                nc.vector.tensor_tensor(out=s2[:], in0=mu[:], in1=s1[:],
                                        op=OP.mult)
                nc.vector.tensor_tensor(out=s2[:],
                                        in0=gb_sb[:, 2 * l + 1:2 * l + 2],
                                        in1=s2[:], op=OP.subtract)

                # ---- normalize (+relu), transpose back to [node, D] ----
                hnorm = bigp.tile([P, cfg.nblk * WBLK], F32, tag="hnorm")
                for b in range(cfg.nblk):
                    blen = cfg.blens[b]
                    sl = slice(b * WBLK, b * WBLK + blen)
                    if l < L - 1:
                        nc.scalar.activation(out=hnorm[:, sl], in_=hpre[:, sl],
                                             func=ACTF.Relu, bias=s2[:, :1],
                                             scale=s1[:, :1])
                    else:
                        nc.vector.tensor_scalar(out=hnorm[:, sl],
                                                in0=hpre[:, sl],
                                                scalar1=s1[:, :1],
                                                scalar2=s2[:, :1],
                                                op0=OP.mult, op1=OP.add)
                if debug_dump:
                    nc.sync.dma_start(out=dbg[f"hnorm{l}"][:, :], in_=hnorm[:])
                for nt in range(cfg.nwin):
                    tl = cfg.winlens[nt]
                    pt = ptp.tile([P, P], F32, tag="pt")
                    nc.tensor.transpose(out=pt[:tl, :],
                                        in_=hnorm[:, nt * P:nt * P + tl],
                                        identity=ident[:])
                    nc.vector.tensor_copy(out=hnew[:tl, nt * P:(nt + 1) * P],
                                          in_=pt[:tl, :])
                    if l < L - 1 and nt == cfg.NFA - 1:
                        # half A written: AllGather it now so the next
                        # layer's phase-A gathers overlap with AG of half B
                        partA = dp.tile([cfg.HA, D], F32, name=f"partA{l}")
                        nc.sync.dma_start(
                            out=partA[:, :].rearrange("(nt p) d -> p nt d",
                                                      p=P),
                            in_=hnew[:, :cfg.NFA * P].rearrange(
                                "p (nt d) -> p nt d", d=D))
                        nc.gpsimd.collective_compute(
                            "AllGather", OP.bypass, replica_groups=rg,
                            ins=[partA[:].opt()], outs=[tabA[l][:].opt()])
                if debug_dump:
                    nc.sync.dma_start(out=dbg[f"hnew{l}"][:, :], in_=hnew[:])

                if l < L - 1:
                    partB = dp.tile([cfg.HB, D], F32, name=f"partB{l}")
                    nfb = cfg.nfull - cfg.NFA
                    if nfb:
                        nc.sync.dma_start(
                            out=partB[:nfb * P, :].rearrange(
                                "(nt p) d -> p nt d", p=P),
                            in_=hnew[:, cfg.NFA * P:cfg.nfull * P].rearrange(
                                "p (nt d) -> p nt d", d=D))
                    if cfg.rem:
                        nc.sync.dma_start(
                            out=partB[nfb * P:, :],
                            in_=hnew[:cfg.rem,
                                     cfg.nfull * P:(cfg.nfull + 1) * P])
                    nc.gpsimd.collective_compute(
                        "AllGather", OP.bypass, replica_groups=rg,
                        ins=[partB[:].opt()], outs=[tabB[l][:].opt()])

            # ---- global mean pool ----
            pgps = pgp.tile([P, G], F32)
            for nt in range(cfg.nwin):
                msel = mselp.tile([P, G], F32, tag="msel")
                nc.vector.tensor_scalar(out=msel[:], in0=iota_sb[:, :G],
                                        scalar1=batchf_sb[:, nt:nt + 1],
                                        scalar2=None, op0=OP.is_equal)
                nc.tensor.matmul(out=pgps[:], lhsT=hnew[:, nt * P:(nt + 1) * P],
                                 rhs=msel[:], start=(nt == 0),
                                 stop=(nt == cfg.nwin - 1))
            poolsb = cp.tile([P, G], F32)
            nc.vector.tensor_copy(out=poolsb[:], in_=pgps[:])
            pool_in = dp.tile([P, G], F32, name="poolin")
            pool_out = dp.tile([P, G], F32, addr_space="Shared", name="poolout")
            nc.sync.dma_start(out=pool_in[:], in_=poolsb[:])
            nc.gpsimd.collective_compute(
                "AllReduce", OP.add, replica_groups=rg,
                ins=[pool_in[:].opt()], outs=[pool_out[:].opt()])
            poolred = cp.tile([P, G], F32)
            nc.sync.dma_start(out=poolred[:], in_=pool_out[:])
            outsb = cp.tile([P, cfg.gblk * D], F32)
            for b in range(cfg.gblk):
                gl = min(P, G - b * P)
                pt = ptp.tile([P, P], F32, tag="pt")
                nc.tensor.transpose(out=pt[:gl, :],
                                    in_=poolred[:, b * P:b * P + gl],
                                    identity=ident[:])
                nc.vector.tensor_scalar(out=outsb[:gl, b * D:(b + 1) * D],
                                        in0=pt[:gl, :],
                                        scalar1=recip_sb[:gl, b:b + 1],
                                        scalar2=None, op0=OP.mult)
            if cfg.gblk == 1:
                nc.sync.dma_start(out=out_e[:, :], in_=outsb[:G, :D])
            else:
                nc.sync.dma_start(
                    out=out_e[:, :].rearrange("(b g) d -> g b d", g=P),
                    in_=outsb[:, :].rearrange("g (b d) -> g b d", d=D))
    nc.compile()
    return nc


_CACHE = {}


def _get_compiled(cfg, sched_key, sched, debug_dump=False):
    key = (cfg.N, cfg.E, cfg.G, cfg.L, sched_key, debug_dump)
    if key not in _CACHE:
        _CACHE[key] = build(cfg, sched, debug_dump=debug_dump)
    return _CACHE[key]


def run(cfg, inputs, trace=False, debug_dump=False):
    shared, per_core, sched = host_preprocess(cfg, **inputs)
    sched_key = (sched["T_lo"], sched["T_hi"],
                 tuple(map(tuple, sched["tiles_wh"])))
    nc = _get_compiled(cfg, sched_key, sched, debug_dump=debug_dump)
    in_maps = [dict(shared, **pc) for pc in per_core]
    res = bass_utils.run_bass_kernel_spmd(
        nc, in_maps, core_ids=list(range(NCORES)), trace=trace)
    out = res.results[0]["out"]
    return out, res


def build_null(cfg, sched):
    """Same external I/O signature as build(), trivial compute — used to
    subtract host/RPC/dispatch overhead from wall-clock timing."""
    T = sched["T"]
    L, N, G = cfg.L, cfg.N, cfg.G
    nc = bacc.Bacc("TRN2", target_bir_lowering=False, debug=False,
                   num_devices=NCORES)
    nc.dram_tensor("xa", [NCORES * cfg.HA, D], F32, kind="ExternalInput")
    nc.dram_tensor("xb", [NCORES * cfg.HB, D], F32, kind="ExternalInput")
    nc.dram_tensor("wt", [P, L * D], F32, kind="ExternalInput")
    nc.dram_tensor("gb", [P, 2 * L], F32, kind="ExternalInput")
    iota_e = nc.dram_tensor("iota", [P, 512], F32, kind="ExternalInput")
    nc.dram_tensor("recip", [P, cfg.gblk], F32, kind="ExternalInput")
    nc.dram_tensor("batchf", [P, cfg.nwin], F32, kind="ExternalInput")
    nc.dram_tensor("idx16", [P, 8 * T], I16, kind="ExternalInput")
    nc.dram_tensor("selmeta", [P, 2 * T], F32, kind="ExternalInput")
    nc.dram_tensor("xown", [P, cfg.nwin * P], F32, kind="ExternalInput")
    nc.dram_tensor("snormpk", [P, cfg.nwin], F32, kind="ExternalInput")
    out_e = nc.dram_tensor("out", [G, D], F32, kind="ExternalOutput")
    with tile.TileContext(nc) as tc:
        with tc.tile_pool(name="sb", bufs=1) as sb:
            t = sb.tile([P, D], F32)
            nc.sync.dma_start(out=t[:], in_=iota_e[:, :D])
            for b in range(-(-G // P)):
                gl = min(P, G - b * P)
                nc.sync.dma_start(out=out_e[b * P:b * P + gl, :],
                                  in_=t[:gl, :])
    nc.compile()
    return nc


def time_pjrt(nc, in_maps, iters=8, warmup=2):
    """Wall-clock repeated executions of the compiled program with
    device-resident inputs (mirrors bass2jax.run_bass_via_pjrt)."""
    import time
    import jax
    from jax.sharding import Mesh, PartitionSpec, NamedSharding
    from jax.experimental.shard_map import shard_map
    from concourse import bass2jax
    from concourse import mybir as mb

    bass2jax.install_neuronx_cc_hook()
    partition_name = (nc.partition_id_tensor.name
                      if nc.partition_id_tensor else None)
    in_names, out_names, out_avals, zero_outs = [], [], [], []
    for alloc in nc.m.functions[0].allocations:
        if not isinstance(alloc, mb.MemoryLocationSet):
            continue
        name = alloc.memorylocations[0].name
        if alloc.kind == "ExternalInput":
            if name != partition_name:
                in_names.append(name)
        elif alloc.kind == "ExternalOutput":
            out_names.append(name)
            shape = tuple(alloc.tensor_shape)
            dtype = mb.dt.np(alloc.dtype)
            out_avals.append(jax.core.ShapedArray(shape, dtype))
            zero_outs.append(np.zeros(shape, dtype))
    n_params = len(in_names)
    in_names = in_names + out_names
    if partition_name is not None:
        in_names.append(partition_name)

    def _body(*args):
        operands = list(args)
        if partition_name is not None:
            operands.append(bass2jax.partition_id_tensor())
        outs = bass2jax._bass_exec_p.bind(
            *operands, out_avals=tuple(out_avals), in_names=tuple(in_names),
            out_names=tuple(out_names), lowering_input_output_aliases=(),
            sim_require_finite=True, sim_require_nnan=True, nc=nc)
        return tuple(outs)

    devices = jax.devices()[:NCORES]
    mesh = Mesh(np.asarray(devices), ("core",))
    spec = PartitionSpec("core")
    in_specs = (spec,) * (n_params + len(out_names))
    out_specs = (spec,) * len(out_names)
    fn = jax.jit(shard_map(_body, mesh=mesh, in_specs=in_specs,
                           out_specs=out_specs, check_rep=False),
                 keep_unused=True)
    sharding = NamedSharding(mesh, spec)
    concat_in = [
        jax.device_put(np.concatenate(
            [np.asarray(in_maps[c][in_names[i]]) for c in range(NCORES)],
            axis=0), sharding)
        for i in range(n_params)
    ]
    concat_zeros = [
        jax.device_put(np.zeros((NCORES * z.shape[0], *z.shape[1:]), z.dtype),
                       sharding)
        for z in zero_outs
    ]
    jax.block_until_ready(concat_in)
    for _ in range(warmup):
        jax.block_until_ready(fn(*concat_in, *concat_zeros))
    times = []
    for _ in range(iters):
        t0 = time.perf_counter()
        jax.block_until_ready(fn(*concat_in, *concat_zeros))
        times.append(time.perf_counter() - t0)
    return times


def measure(cfg, inputs, iters=8):
    """Returns (kernel_walls, null_walls) in seconds."""
    shared, per_core, sched = host_preprocess(cfg, **inputs)
    sched_key = (sched["T_lo"], sched["T_hi"],
                 tuple(map(tuple, sched["tiles_wh"])))
    nc = _get_compiled(cfg, sched_key, sched)
    in_maps = [dict(shared, **pc) for pc in per_core]
    kw = time_pjrt(nc, in_maps, iters=iters)
    key = ("null", cfg.N, cfg.E, cfg.G, sched_key)
    if key not in _CACHE:
        _CACHE[key] = build_null(cfg, sched)
    nw = time_pjrt(_CACHE[key], in_maps, iters=iters)
    return kw, nw


def kernel(x, edge_index, batch, Ws, bs, gammas, betas):
    cfg = Cfg(N=50000, E=625000, G=256, L=3)
    out, _ = run(cfg, dict(x=x, edge_index=edge_index, batch=batch, Ws=Ws,
                           bs=bs, gammas=gammas, betas=betas))
    return out


# revision 29
# speedup vs baseline: 2.0907x; 1.3297x over previous
"""GCN encoder (3-layer GCNConv + BatchNorm + ReLU + global mean pool) on 8
Trainium2 NeuronCores.

Strategy (graph/data parallel, edges sharded by destination):
  - Nodes are split into 8 contiguous shards (one per core). Each core owns
    all edges whose destination lands in its shard.
  - The layer is computed aggregate-first (mathematically identical to the
    reference's transform-first order since GCNConv is linear):
        z[v]  = sum_{e: dst=v} w_e * h[src_e] + snorm_v * h[v]
        hpre  = W.T @ z                          (kept transposed: [D, nodes])
        h_out = relu(gamma * (hpre - mu) / sqrt(var+eps) + beta)
  - The gather h[src_e] uses dma_gather (int16 indices, 2048 rows per
    instruction, single_packet=False) from a replicated node table in HBM.
    The table is stored as two tensors in AllGather order (half A = every
    core's first HA shard rows, half B = the rest) so that (a) each half
    stays under the int16 index range and (b) the next layer's phase-A
    gathers only depend on AG(A), overlapping with AG(B) in flight.
  - Per 128-edge tile, a selection matrix Sel[e, dst_local] = w_e is built
    on-chip with one tensor_scalar(is_equal, mult) against an iota constant,
    and the scatter-add becomes a PE matmul G.T @ Sel accumulated in PSUM
    over a 128-destination window. Self-loops are a diagonal-matrix matmul
    against the previous layer's activations already in SBUF.
  - BatchNorm statistics are free-axis reductions in the transposed layout;
    partials are combined with a [128,2] AllReduce. After normalization the
    result is transposed back (PE transpose) and AllGathered into the next
    layer's node table.
  - Mean pooling reuses the selection-matmul trick against the sorted graph
    ids, followed by a [128,256] AllReduce and division by counts.
"""

import sys

sys.path.insert(0, "/opt/trn_rl_repo")

import numpy as np

import concourse.bass as bass
import concourse.tile as tile
from concourse import bacc, mybir
from concourse import bass_utils
from concourse.masks import make_identity

F32 = mybir.dt.float32
I16 = mybir.dt.int16
OP = mybir.AluOpType
ACTF = mybir.ActivationFunctionType

NCORES = 8
D = 128
P = 128
GB_TILES = 16     # 128-edge tiles per dma_gather
WBLK = 512        # node columns per W-matmul / BN block
EPS = 1e-5


class Cfg:
    def __init__(self, N, E, G, L=3):
        assert N % NCORES == 0
        self.N, self.E, self.G, self.L = N, E, G, L
        self.NP = N // NCORES                    # nodes per core
        self.nwin = -(-self.NP // P)             # 128-dst windows per core
        assert self.nwin >= 2
        # each shard splits into half A (first NFA full node tiles) and
        # half B; the two AllGathers pipeline against the next layer's
        # phase-A gathers
        self.NFA = self.nwin // 2
        self.HA = self.NFA * P
        self.HB = self.NP - self.HA
        assert NCORES * max(self.HA, self.HB) < 32768
        self.winlens = [min(P, self.NP - w * P) for w in range(self.nwin)]
        self.nblk = -(-self.NP // WBLK)          # 512-node BN/W blocks
        self.blens = [min(WBLK, self.NP - b * WBLK) for b in range(self.nblk)]
        self.nfull = self.NP // P                # full 128-node tiles
        self.rem = self.NP - self.nfull * P
        self.gblk = -(-G // P)                   # 128-graph output tiles
        assert self.gblk * P == G or G <= P


def host_preprocess(cfg, x, edge_index, batch, Ws, bs, gammas, betas):
    """Shard + sort edges, build per-core packed metadata arrays."""
    N, G = cfg.N, cfg.G
    NP = cfg.NP
    x = np.ascontiguousarray(np.asarray(x, np.float32))
    src = np.asarray(edge_index[0]).astype(np.int64)
    dst = np.asarray(edge_index[1]).astype(np.int64)
    batch = np.asarray(batch).astype(np.int64)

    deg = (1.0 + np.bincount(dst, minlength=N)).astype(np.float32)
    dis = (1.0 / np.sqrt(deg)).astype(np.float32)
    enorm = dis[src] * dis[dst]
    snorm = (dis * dis).astype(np.float32)

    counts = np.bincount(batch, minlength=G).astype(np.float32)
    recip = (1.0 / np.maximum(counts, 1.0)).astype(np.float32)

    # per-core edge lists sharded by dst, sorted by (half, local dst);
    # the gather table is stored in AllGather order: half A = concat of all
    # cores' first HA rows, half B = concat of the rest
    per_core = []
    core_of = dst // NP
    for c in range(NCORES):
        m = core_of == c
        s, dl, w = src[m], dst[m] - c * NP, enorm[m]
        sc = s // NP
        sl = s - sc * NP
        h = (sl >= cfg.HA).astype(np.int64)
        rel = np.where(h == 0, sc * cfg.HA + sl, sc * cfg.HB + (sl - cfg.HA))
        order = np.lexsort((dl, h))
        per_core.append((rel[order], dl[order], w[order], h[order]))

    # shared static tile schedule: per (window, half), max tiles over cores
    nwin = cfg.nwin
    cnt = np.zeros((NCORES, nwin, 2), np.int64)
    bounds = []
    for c in range(NCORES):
        s, dl, w, h = per_core[c]
        nlo = int(np.searchsorted(h, 1))
        blo = np.searchsorted(dl[:nlo], np.arange(nwin + 1) * P)
        bhi = nlo + np.searchsorted(dl[nlo:], np.arange(nwin + 1) * P)
        bounds.append((blo, bhi))
        cnt[c, :, 0] = blo[1:] - blo[:-1]
        cnt[c, :, 1] = bhi[1:] - bhi[:-1]
    tiles_wh = np.max(-(-cnt // P), axis=0)      # [nwin, 2]
    T_lo = int(tiles_wh[:, 0].sum())
    T_hi = int(tiles_wh[:, 1].sum())
    T = T_lo + T_hi

    src_rel = np.zeros((NCORES, P, T), np.int16)
    selmeta = np.zeros((NCORES, P, 2 * T), np.float32)
    for c in range(NCORES):
        s, dl, w, h = per_core[c]
        blo, bhi = bounds[c]
        for half in (0, 1):
            t0 = 0 if half == 0 else T_lo
            bb = blo if half == 0 else bhi
            for wi in range(nwin):
                for j in range(int(tiles_wh[wi, half])):
                    a = bb[wi] + j * P
                    n = max(0, min(P, bb[wi + 1] - a))
                    t = t0 + j
                    if n > 0:
                        src_rel[c, :n, t] = s[a:a + n]
                        selmeta[c, :n, 2 * t] = (dl[a:a + n] - wi * P)
                        selmeta[c, :n, 2 * t + 1] = w[a:a + n]
                t0 += int(tiles_wh[wi, half])

    # int16 index stream for dma_gather: flat position i -> [i%16, i//16],
    # replicated across the 8 16-partition groups
    idx16 = np.zeros((NCORES, P, 8 * T), np.int16)
    for c in range(NCORES):
        flat = src_rel[c].T.reshape(-1)          # tile-major, then partition
        wrapped = flat.reshape(-1, 16).T         # [16, 8*T]
        idx16[c] = np.tile(wrapped, (8, 1))

    # per-core own-shard features in [node%128, tile*128+d] layout (layer-0
    # self-loop operand), zero-padded tail
    NT = nwin
    x_own = np.zeros((NCORES, P, NT * P), np.float32)
    snorm_pk = np.zeros((NCORES, P, NT), np.float32)
    batchf = np.full((NCORES, P, NT), -1.0, np.float32)
    for c in range(NCORES):
        xs = x[c * NP:(c + 1) * NP]
        pad = np.zeros((NT * P - NP, D), np.float32)
        x_own[c] = np.concatenate([xs, pad]).reshape(NT, P, D).transpose(
            1, 0, 2).reshape(P, NT * P)
        sn = np.concatenate([snorm[c * NP:(c + 1) * NP],
                             np.zeros(NT * P - NP, np.float32)])
        snorm_pk[c] = sn.reshape(NT, P).T
        ids = np.concatenate([batch[c * NP:(c + 1) * NP].astype(np.float32),
                              np.full(NT * P - NP, -1.0, np.float32)])
        batchf[c] = ids.reshape(NT, P).T

    iota = np.broadcast_to(np.arange(512, dtype=np.float32), (P, 512)).copy()
    Wpack = np.asarray(Ws, np.float32).transpose(1, 0, 2).reshape(D, cfg.L * D)
    gb = np.zeros((P, 2 * cfg.L), np.float32)
    for l in range(cfg.L):
        gb[:, 2 * l] = np.asarray(gammas[l], np.float32)
        gb[:, 2 * l + 1] = np.asarray(betas[l], np.float32)
    recip_pk = np.zeros((P, cfg.gblk), np.float32)
    for b in range(cfg.gblk):
        n = min(P, G - b * P)
        recip_pk[:n, b] = recip[b * P:b * P + n]

    xa = np.concatenate([x[c * NP:c * NP + cfg.HA] for c in range(NCORES)])
    xb = np.concatenate([x[c * NP + cfg.HA:(c + 1) * NP]
                         for c in range(NCORES)])
    shared = dict(xa=np.ascontiguousarray(xa), xb=np.ascontiguousarray(xb),
                  wt=Wpack, gb=gb, iota=iota, recip=recip_pk)
    per_core_inputs = [dict(idx16=np.ascontiguousarray(idx16[c]),
                            selmeta=np.ascontiguousarray(selmeta[c]),
                            batchf=np.ascontiguousarray(batchf[c]),
                            xown=np.ascontiguousarray(x_own[c]),
                            snormpk=np.ascontiguousarray(snorm_pk[c]))
                       for c in range(NCORES)]
    sched = dict(tiles_wh=tiles_wh, T_lo=T_lo, T_hi=T_hi, T=T)
    return shared, per_core_inputs, sched


def build(cfg, sched, debug_dump=False):
    tiles_wh = sched["tiles_wh"]
    T_lo, T_hi, T = sched["T_lo"], sched["T_hi"], sched["T"]
    L, N, G, NP = cfg.L, cfg.N, cfg.G, cfg.NP

    nc = bacc.Bacc("TRN2", target_bir_lowering=False, debug=False,
                   num_devices=NCORES)
    dbg = {}
    if debug_dump:
        for nm, shape in [("zT", [P, cfg.nwin * P]),
                          ("hpre", [P, cfg.nblk * WBLK]),
                          ("stat", [P, 2]),
                          ("hnorm", [P, cfg.nblk * WBLK]),
                          ("hnew", [P, cfg.nwin * P]),
                          ("gdump", [P, GB_TILES * P])]:
            for l in range(L):
                dbg[f"{nm}{l}"] = nc.dram_tensor(
                    f"dbg_{nm}{l}", shape, F32, kind="ExternalOutput")

    xa_e = nc.dram_tensor("xa", [NCORES * cfg.HA, D], F32,
                          kind="ExternalInput")
    xb_e = nc.dram_tensor("xb", [NCORES * cfg.HB, D], F32,
                          kind="ExternalInput")
    wt_e = nc.dram_tensor("wt", [P, L * D], F32, kind="ExternalInput")
    gb_e = nc.dram_tensor("gb", [P, 2 * L], F32, kind="ExternalInput")
    iota_e = nc.dram_tensor("iota", [P, 512], F32, kind="ExternalInput")
    recip_e = nc.dram_tensor("recip", [P, cfg.gblk], F32, kind="ExternalInput")
    batchf_e = nc.dram_tensor("batchf", [P, cfg.nwin], F32, kind="ExternalInput")
    idx16_e = nc.dram_tensor("idx16", [P, 8 * T], I16, kind="ExternalInput")
    selmeta_e = nc.dram_tensor("selmeta", [P, 2 * T], F32, kind="ExternalInput")
    xown_e = nc.dram_tensor("xown", [P, cfg.nwin * P], F32, kind="ExternalInput")
    snorm_e = nc.dram_tensor("snormpk", [P, cfg.nwin], F32, kind="ExternalInput")
    out_e = nc.dram_tensor("out", [G, D], F32, kind="ExternalOutput")

    rg = [list(range(NCORES))]

    with tile.TileContext(nc) as tc:
        with tc.tile_pool(name="const", bufs=1) as cp, \
             tc.tile_pool(name="gpool", bufs=3) as gp, \
             tc.tile_pool(name="selp", bufs=8) as selp, \
             tc.tile_pool(name="mselp", bufs=3) as mselp, \
             tc.tile_pool(name="big", bufs=1) as bigp, \
             tc.tile_pool(name="scr", bufs=2) as scrp, \
             tc.tile_pool(name="small", bufs=4) as smp, \
             tc.tile_pool(name="pz", bufs=3, space="PSUM") as pzp, \
             tc.tile_pool(name="ph", bufs=2, space="PSUM") as php, \
             tc.tile_pool(name="pt", bufs=2, space="PSUM") as ptp, \
             tc.tile_pool(name="pg", bufs=1, space="PSUM") as pgp, \
             tc.tile_pool(name="dram", bufs=1, space="DRAM") as dp:

            # ---- constants into SBUF ----
            iota_sb = cp.tile([P, 512], F32)
            nc.sync.dma_start(out=iota_sb[:], in_=iota_e[:, :])
            wt_sb = cp.tile([P, L * D], F32)
            nc.sync.dma_start(out=wt_sb[:], in_=wt_e[:, :])
            gb_sb = cp.tile([P, 2 * L], F32)
            nc.sync.dma_start(out=gb_sb[:], in_=gb_e[:, :])
            recip_sb = cp.tile([P, cfg.gblk], F32)
            nc.sync.dma_start(out=recip_sb[:], in_=recip_e[:, :])
            batchf_sb = cp.tile([P, cfg.nwin], F32)
            nc.sync.dma_start(out=batchf_sb[:], in_=batchf_e[:, :])
            idx_sb = cp.tile([P, 8 * T], I16)
            nc.sync.dma_start(out=idx_sb[:], in_=idx16_e[:, :])
            meta_sb = cp.tile([P, 2 * T], F32)
            nc.sync.dma_start(out=meta_sb[:], in_=selmeta_e[:, :])
            xown_sb = cp.tile([P, cfg.nwin * P], F32)
            nc.sync.dma_start(out=xown_sb[:], in_=xown_e[:, :])
            snorm_sb = cp.tile([P, cfg.nwin], F32)
            nc.sync.dma_start(out=snorm_sb[:], in_=snorm_e[:, :])
            ident = cp.tile([P, P], F32)
            make_identity(nc, ident[:])
            zero_c = cp.tile([P, 1], F32)
            nc.vector.memset(zero_c[:], 0.0)
            eps_c = cp.tile([P, 1], F32)
            nc.vector.memset(eps_c[:], EPS)

            hnew = bigp.tile([P, cfg.nwin * P], F32)
            if cfg.rem:
                # matmuls read the full 128 partitions of the last node
                # tile; make the unwritten tail well-defined zeros (runs
                # once; per-layer transposes overwrite only valid rows)
                nc.vector.memset(hnew[:, cfg.nfull * P:], 0.0)

            tabA, tabB = [], []
            for l in range(L - 1):
                tabA.append(dp.tile([NCORES * cfg.HA, D], F32,
                                    addr_space="Shared", name=f"tabA{l}"))
                tabB.append(dp.tile([NCORES * cfg.HB, D], F32,
                                    addr_space="Shared", name=f"tabB{l}"))

            # per-(window,half) phase-local first tile index
            starts = np.zeros((cfg.nwin, 2), np.int64)
            t0 = 0
            for w in range(cfg.nwin):
                starts[w, 0] = t0
                t0 += int(tiles_wh[w][0])
            t0 = 0
            for w in range(cfg.nwin):
                starts[w, 1] = t0
                t0 += int(tiles_wh[w][1])

            for l in range(L):
                srcA = xa_e if l == 0 else tabA[l - 1]
                srcB = xb_e if l == 0 else tabB[l - 1]
                hprev = xown_sb if l == 0 else hnew

                zT = bigp.tile([P, cfg.nwin * P], F32, tag="zT")
                gather_tiles = [{}, {}]

                def ensure_gather(half, t_local, l=l, srcA=srcA,
                                  srcB=srcB, gather_tiles=gather_tiles):
                    """Issue the dma_gather covering phase-local tile t_local
                    of the given half; returns the gather SBUF tile."""
                    k = t_local // GB_TILES
                    cache = gather_tiles[half]
                    if k not in cache:
                        T_ph = T_lo if half == 0 else T_hi
                        cnt_t = min(GB_TILES, T_ph - k * GB_TILES)
                        g = gp.tile([P, GB_TILES * P], F32, tag="gath")
                        base = (0 if half == 0 else T_lo) + k * GB_TILES
                        tbl = srcA[:, :] if half == 0 else srcB[:, :]
                        nc.gpsimd.dma_gather(
                            out_ap=g[:, :cnt_t * P].rearrange(
                                "p (t d) -> p t d", d=P),
                            in_ap=tbl,
                            idxs_ap=idx_sb[:, base * 8:(base + cnt_t) * 8],
                            num_idxs=cnt_t * P,
                            num_idxs_reg=cnt_t * P,
                            elem_size=D,
                            single_packet=False,
                        )
                        if debug_dump and half == 0 and k == 0:
                            nc.sync.dma_start(out=dbg[f"gdump{l}"][:, :],
                                              in_=g[:])
                        cache[k] = g
                    return cache[k]

                # --- lo phase: self-loop diagonal + lo-half edge tiles ---
                for w in range(cfg.nwin):
                    wlen = cfg.winlens[w]
                    nlo = int(tiles_wh[w][0])
                    pz = pzp.tile([P, P], F32, tag="pz")
                    dg = selp.tile([P, P], F32, tag="sel")
                    nc.vector.tensor_scalar(
                        out=dg[:, :wlen], in0=ident[:, :wlen],
                        scalar1=snorm_sb[:, w:w + 1], scalar2=None,
                        op0=OP.mult)
                    nc.tensor.matmul(out=pz[:, :wlen],
                                     lhsT=hprev[:, w * P:(w + 1) * P],
                                     rhs=dg[:, :wlen],
                                     start=True, stop=(nlo == 0))
                    for j in range(nlo):
                        t = int(starts[w, 0]) + j
                        g = ensure_gather(0, t)
                        slot = t % GB_TILES
                        sel = selp.tile([P, P], F32, tag="sel")
                        nc.vector.tensor_scalar(
                            out=sel[:, :wlen], in0=iota_sb[:, :wlen],
                            scalar1=meta_sb[:, 2 * t:2 * t + 1],
                            scalar2=meta_sb[:, 2 * t + 1:2 * t + 2],
                            op0=OP.is_equal, op1=OP.mult)
                        nc.tensor.matmul(out=pz[:, :wlen],
                                         lhsT=g[:, slot * P:(slot + 1) * P],
                                         rhs=sel[:, :wlen],
                                         start=False, stop=(j == nlo - 1))
                    nc.vector.tensor_copy(out=zT[:, w * P:w * P + wlen],
                                          in_=pz[:, :wlen])

                # --- hi phase: hi-half edge tiles, added into zT ---
                for w in range(cfg.nwin):
                    wlen = cfg.winlens[w]
                    nhi = int(tiles_wh[w][1])
                    if nhi == 0:
                        continue
                    pz = pzp.tile([P, P], F32, tag="pz")
                    for j in range(nhi):
                        t = int(starts[w, 1]) + j
                        g = ensure_gather(1, t)
                        slot = t % GB_TILES
                        tm = T_lo + t
                        sel = selp.tile([P, P], F32, tag="sel")
                        nc.vector.tensor_scalar(
                            out=sel[:, :wlen], in0=iota_sb[:, :wlen],
                            scalar1=meta_sb[:, 2 * tm:2 * tm + 1],
                            scalar2=meta_sb[:, 2 * tm + 1:2 * tm + 2],
                            op0=OP.is_equal, op1=OP.mult)
                        nc.tensor.matmul(out=pz[:, :wlen],
                                         lhsT=g[:, slot * P:(slot + 1) * P],
                                         rhs=sel[:, :wlen],
                                         start=(j == 0), stop=(j == nhi - 1))
                    nc.vector.tensor_tensor(out=zT[:, w * P:w * P + wlen],
                                            in0=zT[:, w * P:w * P + wlen],
                                            in1=pz[:, :wlen], op=OP.add)

                if debug_dump:
                    nc.sync.dma_start(out=dbg[f"zT{l}"][:, :], in_=zT[:])

                # ---- W matmul + BN stats ----
                hpre = bigp.tile([P, cfg.nblk * WBLK], F32, tag="hpre")
                sums = smp.tile([P, cfg.nblk], F32, tag="sums")
                sqs = smp.tile([P, cfg.nblk], F32, tag="sqs")
                for b in range(cfg.nblk):
                    blen = cfg.blens[b]
                    ph = php.tile([P, WBLK], F32, tag="ph")
                    nc.tensor.matmul(
                        out=ph[:, :blen],
                        lhsT=wt_sb[:, l * D:(l + 1) * D],
                        rhs=zT[:, b * WBLK:b * WBLK + blen],
                        start=True, stop=True)
                    nc.scalar.activation(
                        out=hpre[:, b * WBLK:b * WBLK + blen],
                        in_=ph[:, :blen], func=ACTF.Copy,
                        accum_out=sums[:, b:b + 1])
                    scr = scrp.tile([P, WBLK], F32, tag="scr")
                    nc.scalar.activation(
                        out=scr[:, :blen], in_=ph[:, :blen], func=ACTF.Square,
                        bias=zero_c[:, :1], accum_out=sqs[:, b:b + 1])

                ssum = smp.tile([P, 1], F32, tag="ssum")
                ssq = smp.tile([P, 1], F32, tag="ssq")
                nc.vector.reduce_sum(out=ssum[:], in_=sums[:],
                                     axis=mybir.AxisListType.X)
                nc.vector.reduce_sum(out=ssq[:], in_=sqs[:],
                                     axis=mybir.AxisListType.X)
                statpk = smp.tile([P, 2], F32, tag="statpk")
                nc.vector.tensor_copy(out=statpk[:, 0:1], in_=ssum[:])
                nc.vector.tensor_copy(out=statpk[:, 1:2], in_=ssq[:])
                stat_in = dp.tile([P, 2], F32, name=f"statin{l}")
                stat_out = dp.tile([P, 2], F32, addr_space="Shared",
                                   name=f"statout{l}")
                nc.sync.dma_start(out=stat_in[:], in_=statpk[:])
                nc.gpsimd.collective_compute(
                    "AllReduce", OP.add, replica_groups=rg,
                    ins=[stat_in[:].opt()], outs=[stat_out[:].opt()])
                statred = smp.tile([P, 2], F32, tag="statred")
                nc.sync.dma_start(out=statred[:], in_=stat_out[:])

                if debug_dump:
                    nc.sync.dma_start(out=dbg[f"hpre{l}"][:, :], in_=hpre[:])
                    nc.sync.dma_start(out=dbg[f"stat{l}"][:, :], in_=statred[:])

                mu = smp.tile([P, 1], F32, tag="mu")
                ex2 = smp.tile([P, 1], F32, tag="ex2")
                var = smp.tile([P, 1], F32, tag="var")
                std = smp.tile([P, 1], F32, tag="std")
                rsinv = smp.tile([P, 1], F32, tag="rsinv")
                s1 = smp.tile([P, 1], F32, tag="s1")
                s2 = smp.tile([P, 1], F32, tag="s2")
                inv_n = float(np.float32(1.0 / N))
                nc.vector.tensor_scalar(out=mu[:], in0=statred[:, 0:1],
                                        scalar1=inv_n, scalar2=None,
                                        op0=OP.mult)
                nc.vector.tensor_scalar(out=ex2[:], in0=statred[:, 1:2],
                                        scalar1=inv_n, scalar2=None,
                                        op0=OP.mult)
                nc.vector.scalar_tensor_tensor(
                    out=var[:], in0=mu[:], scalar=1.0, in1=mu[:],
                    op0=OP.bypass, op1=OP.mult)
                nc.vector.tensor_tensor(out=var[:], in0=ex2[:], in1=var[:],
                                        op=OP.subtract)
                nc.scalar.activation(out=std[:], in_=var[:], func=ACTF.Sqrt,
                                     bias=eps_c[:, :1])
                nc.vector.reciprocal(out=rsinv[:], in_=std[:])
                nc.vector.tensor_tensor(out=s1[:], in0=gb_sb[:, 2 * l:2 * l + 1],
                                        in1=rsinv[:], op=OP.mult)
                nc.vector.tensor_tensor(out=s2[:], in0=mu[:], in1=s1[:],
                                        op=OP.mult)
                nc.vector.tensor_tensor(out=s2[:],
                                        in0=gb_sb[:, 2 * l + 1:2 * l + 2],
                                        in1=s2[:], op=OP.subtract)

                # ---- normalize (+relu), transpose back to [node, D] ----
                hnorm = bigp.tile([P, cfg.nblk * WBLK], F32, tag="hnorm")
                for b in range(cfg.nblk):
                    blen = cfg.blens[b]
                    sl = slice(b * WBLK, b * WBLK + blen)
                    if l < L - 1:
                        nc.scalar.activation(out=hnorm[:, sl], in_=hpre[:, sl],
                                             func=ACTF.Relu, bias=s2[:, :1],
                                             scale=s1[:, :1])
                    else:
                        nc.vector.tensor_scalar(out=hnorm[:, sl],
                                                in0=hpre[:, sl],
                                                scalar1=s1[:, :1],
                                                scalar2=s2[:, :1],
                                                op0=OP.mult, op1=OP.add)
                if debug_dump:
                    nc.sync.dma_start(out=dbg[f"hnorm{l}"][:, :], in_=hnorm[:])
                for nt in range(cfg.nwin):
                    tl = cfg.winlens[nt]
                    pt = ptp.tile([P, P], F32, tag="pt")
                    nc.tensor.transpose(out=pt[:tl, :],
                                        in_=hnorm[:, nt * P:nt * P + tl],
                                        identity=ident[:])
                    nc.vector.tensor_copy(out=hnew[:tl, nt * P:(nt + 1) * P],
                                          in_=pt[:tl, :])
                    if l < L - 1 and nt == cfg.NFA - 1:
                        # half A written: AllGather it now so the next
                        # layer's phase-A gathers overlap with AG of half B
                        partA = dp.tile([cfg.HA, D], F32, name=f"partA{l}")
                        nc.sync.dma_start(
                            out=partA[:, :].rearrange("(nt p) d -> p nt d",
                                                      p=P),
                            in_=hnew[:, :cfg.NFA * P].rearrange(
                                "p (nt d) -> p nt d", d=D))
                        nc.gpsimd.collective_compute(
                            "AllGather", OP.bypass, replica_groups=rg,
                            ins=[partA[:].opt()], outs=[tabA[l][:].opt()])
                if debug_dump:
                    nc.sync.dma_start(out=dbg[f"hnew{l}"][:, :], in_=hnew[:])

                if l < L - 1:
                    partB = dp.tile([cfg.HB, D], F32, name=f"partB{l}")
                    nfb = cfg.nfull - cfg.NFA
                    if nfb:
                        nc.sync.dma_start(
                            out=partB[:nfb * P, :].rearrange(
                                "(nt p) d -> p nt d", p=P),
                            in_=hnew[:, cfg.NFA * P:cfg.nfull * P].rearrange(
                                "p (nt d) -> p nt d", d=D))
                    if cfg.rem:
                        nc.sync.dma_start(
                            out=partB[nfb * P:, :],
                            in_=hnew[:cfg.rem,
                                     cfg.nfull * P:(cfg.nfull + 1) * P])
                    nc.gpsimd.collective_compute(
                        "AllGather", OP.bypass, replica_groups=rg,
                        ins=[partB[:].opt()], outs=[tabB[l][:].opt()])

            # ---- global mean pool ----
            pgps = pgp.tile([P, G], F32)
            for nt in range(cfg.nwin):
                msel = mselp.tile([P, G], F32, tag="msel")
                nc.vector.tensor_scalar(out=msel[:], in0=iota_sb[:, :G],
                                        scalar1=batchf_sb[:, nt:nt + 1],
                                        scalar2=None, op0=OP.is_equal)
                nc.tensor.matmul(out=pgps[:], lhsT=hnew[:, nt * P:(nt + 1) * P],
                                 rhs=msel[:], start=(nt == 0),
                                 stop=(nt == cfg.nwin - 1))
            poolsb = cp.tile([P, G], F32)
            nc.vector.tensor_copy(out=poolsb[:], in_=pgps[:])
            pool_in = dp.tile([P, G], F32, name="poolin")
            pool_out = dp.tile([P, G], F32, addr_space="Shared", name="poolout")
            nc.sync.dma_start(out=pool_in[:], in_=poolsb[:])
            nc.gpsimd.collective_compute(
                "AllReduce", OP.add, replica_groups=rg,
                ins=[pool_in[:].opt()], outs=[pool_out[:].opt()])
            poolred = cp.tile([P, G], F32)
            nc.sync.dma_start(out=poolred[:], in_=pool_out[:])
            outsb = cp.tile([P, cfg.gblk * D], F32)
            for b in range(cfg.gblk):
                gl = min(P, G - b * P)
                pt = ptp.tile([P, P], F32, tag="pt")
                nc.tensor.transpose(out=pt[:gl, :],
                                    in_=poolred[:, b * P:b * P + gl],
                                    identity=ident[:])
                nc.vector.tensor_scalar(out=outsb[:gl, b * D:(b + 1) * D],
                                        in0=pt[:gl, :],
                                        scalar1=recip_sb[:gl, b:b + 1],
                                        scalar2=None, op0=OP.mult)
            if cfg.gblk == 1:
                nc.sync.dma_start(out=out_e[:, :], in_=outsb[:G, :D])
            else:
                nc.sync.dma_start(
                    out=out_e[:, :].rearrange("(b g) d -> g b d", g=P),
                    in_=outsb[:, :].rearrange("g (b d) -> g b d", d=D))
    nc.compile()
    return nc


_CACHE = {}


def _get_compiled(cfg, sched_key, sched, debug_dump=False):
    key = (cfg.N, cfg.E, cfg.G, cfg.L, sched_key, debug_dump)
    if key not in _CACHE:
        _CACHE[key] = build(cfg, sched, debug_dump=debug_dump)
    return _CACHE[key]


def run(cfg, inputs, trace=False, debug_dump=False):
    shared, per_core, sched = host_preprocess(cfg, **inputs)
    sched_key = (sched["T_lo"], sched["T_hi"],
                 tuple(map(tuple, sched["tiles_wh"])))
    nc = _get_compiled(cfg, sched_key, sched, debug_dump=debug_dump)
    in_maps = [dict(shared, **pc) for pc in per_core]
    res = bass_utils.run_bass_kernel_spmd(
        nc, in_maps, core_ids=list(range(NCORES)), trace=trace)
    out = res.results[0]["out"]
    return out, res


def build_null(cfg, sched):
    """Same external I/O signature as build(), trivial compute — used to
    subtract host/RPC/dispatch overhead from wall-clock timing."""
    T = sched["T"]
    L, N, G = cfg.L, cfg.N, cfg.G
    nc = bacc.Bacc("TRN2", target_bir_lowering=False, debug=False,
                   num_devices=NCORES)
    nc.dram_tensor("xa", [NCORES * cfg.HA, D], F32, kind="ExternalInput")
    nc.dram_tensor("xb", [NCORES * cfg.HB, D], F32, kind="ExternalInput")
    nc.dram_tensor("wt", [P, L * D], F32, kind="ExternalInput")
    nc.dram_tensor("gb", [P, 2 * L], F32, kind="ExternalInput")
    iota_e = nc.dram_tensor("iota", [P, 512], F32, kind="ExternalInput")
    nc.dram_tensor("recip", [P, cfg.gblk], F32, kind="ExternalInput")
    nc.dram_tensor("batchf", [P, cfg.nwin], F32, kind="ExternalInput")
    nc.dram_tensor("idx16", [P, 8 * T], I16, kind="ExternalInput")
    nc.dram_tensor("selmeta", [P, 2 * T], F32, kind="ExternalInput")
    nc.dram_tensor("xown", [P, cfg.nwin * P], F32, kind="ExternalInput")
    nc.dram_tensor("snormpk", [P, cfg.nwin], F32, kind="ExternalInput")
    out_e = nc.dram_tensor("out", [G, D], F32, kind="ExternalOutput")
    with tile.TileContext(nc) as tc:
        with tc.tile_pool(name="sb", bufs=1) as sb:
            t = sb.tile([P, D], F32)
            nc.sync.dma_start(out=t[:], in_=iota_e[:, :D])
            for b in range(-(-G // P)):
                gl = min(P, G - b * P)
                nc.sync.dma_start(out=out_e[b * P:b * P + gl, :],
                                  in_=t[:gl, :])
    nc.compile()
    return nc


def time_pjrt(nc, in_maps, iters=8, warmup=2):
    """Wall-clock repeated executions of the compiled program with
    device-resident inputs (mirrors bass2jax.run_bass_via_pjrt)."""
    import time
    import jax
    from jax.sharding import Mesh, PartitionSpec, NamedSharding
    from jax.experimental.shard_map import shard_map
    from concourse import bass2jax
    from concourse import mybir as mb

    bass2jax.install_neuronx_cc_hook()
    partition_name = (nc.partition_id_tensor.name
                      if nc.partition_id_tensor else None)
    in_names, out_names, out_avals, zero_outs = [], [], [], []
    for alloc in nc.m.functions[0].allocations:
        if not isinstance(alloc, mb.MemoryLocationSet):
            continue
        name = alloc.memorylocations[0].name
        if alloc.kind == "ExternalInput":
            if name != partition_name:
                in_names.append(name)
        elif alloc.kind == "ExternalOutput":
            out_names.append(name)
            shape = tuple(alloc.tensor_shape)
            dtype = mb.dt.np(alloc.dtype)
            out_avals.append(jax.core.ShapedArray(shape, dtype))
            zero_outs.append(np.zeros(shape, dtype))
    n_params = len(in_names)
    in_names = in_names + out_names
    if partition_name is not None:
        in_names.append(partition_name)

    def _body(*args):
        operands = list(args)
        if partition_name is not None:
            operands.append(bass2jax.partition_id_tensor())
        outs = bass2jax._bass_exec_p.bind(
            *operands, out_avals=tuple(out_avals), in_names=tuple(in_names),
            out_names=tuple(out_names), lowering_input_output_aliases=(),
            sim_require_finite=True, sim_require_nnan=True, nc=nc)
        return tuple(outs)

    devices = jax.devices()[:NCORES]
    mesh = Mesh(np.asarray(devices), ("core",))
    spec = PartitionSpec("core")
    in_specs = (spec,) * (n_params + len(out_names))
    out_specs = (spec,) * len(out_names)
    fn = jax.jit(shard_map(_body, mesh=mesh, in_specs=in_specs,
                           out_specs=out_specs, check_rep=False),
                 keep_unused=True)
    sharding = NamedSharding(mesh, spec)
    concat_in = [
        jax.device_put(np.concatenate(
            [np.asarray(in_maps[c][in_names[i]]) for c in range(NCORES)],
            axis=0), sharding)
        for i in range(n_params)
    ]
    concat_zeros = [
        jax.device_put(np.zeros((NCORES * z.shape[0], *z.shape[1:]), z.dtype),
                       sharding)
        for z in zero_outs
    ]
    jax.block_until_ready(concat_in)
    for _ in range(warmup):
        jax.block_until_ready(fn(*concat_in, *concat_zeros))
    times = []
    for _ in range(iters):
        t0 = time.perf_counter()
        jax.block_until_ready(fn(*concat_in, *concat_zeros))
        times.append(time.perf_counter() - t0)
    return times


def measure(cfg, inputs, iters=8):
    """Returns (kernel_walls, null_walls) in seconds."""
    shared, per_core, sched = host_preprocess(cfg, **inputs)
    sched_key = (sched["T_lo"], sched["T_hi"],
                 tuple(map(tuple, sched["tiles_wh"])))
    nc = _get_compiled(cfg, sched_key, sched)
    in_maps = [dict(shared, **pc) for pc in per_core]
    kw = time_pjrt(nc, in_maps, iters=iters)
    key = ("null", cfg.N, cfg.E, cfg.G, sched_key)
    if key not in _CACHE:
        _CACHE[key] = build_null(cfg, sched)
    nw = time_pjrt(_CACHE[key], in_maps, iters=iters)
    return kw, nw


def kernel(x, edge_index, batch, Ws, bs, gammas, betas):
    cfg = Cfg(N=50000, E=625000, G=256, L=3)
    out, _ = run(cfg, dict(x=x, edge_index=edge_index, batch=batch, Ws=Ws,
                           bs=bs, gammas=gammas, betas=betas))
    return out
